# Initial kernel scaffold
#
"""Trainium2 Bass kernel for GAT-style GNN message passing (edge softmax).

Contract: kernel(**inputs) takes FULL unsharded numpy inputs (see shapes
below), distributes across 8 NeuronCores internally, returns FULL output.

Sharding: edges sorted by dst and partitioned by dst range (6250 nodes per
core) -> every per-destination segment reduction is core-local (no
all-reduce of partial sums). Node features/weights replicated; q|v computed
sharded and exchanged with one AllGather. Segment sums are computed as
one-hot matmuls accumulated in PSUM; the softmax max-subtraction is
algebraically unnecessary here (logits bounded by (5 + |e_bias|)*4 << 88).
"""

import os
import sys

sys.path.insert(0, "/opt/trn_rl_repo")

import numpy as np

import concourse.bass as bass
import concourse.mybir as mybir
import concourse.tile as tile
from concourse import bacc
from concourse import bass_utils
from concourse.masks import make_identity

F32 = mybir.dt.float32
F16 = mybir.dt.float16
F8 = mybir.dt.float8e4
I32 = mybir.dt.int32
AF = mybir.ActivationFunctionType
OP = mybir.AluOpType

D = 128
H = 8
HD = 16
EPS = 1e-5

# Full-problem config (hardcoded per problem spec).
N_NODES = 50000
N_EDGES = 800000
CORES = 8
NPC = N_NODES // CORES      # nodes per core = 6250
BLK = 125                   # dst nodes per block (<=125 so cols 125..127 discard pads)
NBLK = NPC // BLK           # 50 blocks per core

# Edge payload dtype for the gathered tables (q|v, k) and eg intermediates.
# "f32" is exact; "f16" halves gather traffic.
EDGE_DT = os.environ.get("KERNEL_EDGE_DT", "f16")


def _edge_dt():
    return F16 if EDGE_DT == "f16" else F32


def _force_act_set():
    """Pin every ACTIVATE to the natural_log_exp_and_others table so the
    kernel pays one ACT_TABLE_LOAD instead of hundreds (sets are chosen by
    bacc's insert_act_table_loads from this mapping)."""
    from concourse import hw_specs

    if getattr(bacc, "_act_set_forced", False):
        return
    real = hw_specs.get_activation_tables

    def patched(arch):
        t = dict(real(arch))
        keep = "natural_log_exp_and_others"
        return {name: (fns if name == keep else set()) for name, fns in t.items()}

    bacc.get_activation_tables = patched
    bacc._act_set_forced = True


def build_program(cfg):
    """Build the SPMD Bass/Tile program. cfg keys:
    cores, n_nodes, npc, nblk, blk, M
    """
    _force_act_set()
    cores = cfg["cores"]
    n_nodes = cfg["n_nodes"]
    npc = cfg["npc"]
    nblk = cfg["nblk"]
    blk = cfg["blk"]
    M = cfg["M"]
    HM = cfg["HM"]
    CAP = M * 128
    HCAP = HM * 128
    LCAP = CAP - HCAP
    EPC = nblk * CAP
    edt = _edge_dt()

    nc = bacc.Bacc(
        "TRN2", target_bir_lowering=False, debug=False, num_devices=cores
    )

    # ---- I/O ----
    eft_d = nc.dram_tensor("eft", [D, EPC], F32, kind="ExternalInput").ap()
    fslotT_d = nc.dram_tensor("fslotT", [D, EPC], F32, kind="ExternalInput").ap()
    dloc_d = nc.dram_tensor("dloc", [nblk, 128, M], I32, kind="ExternalInput").ap()
    ohT_d = nc.dram_tensor("ohT", [nblk, 128, M, 128], F8, kind="ExternalInput").ap()
    featT_d = nc.dram_tensor("featT", [D, npc], F32, kind="ExternalInput").ap()
    w_in = {}
    for name in ("Wq", "Wk", "Wv", "Wo", "Wskip", "W1", "W2", "WoT", "WskipT"):
        w_in[name] = nc.dram_tensor(name, [D, D], F32, kind="ExternalInput").ap()
    w_in["We"] = nc.dram_tensor("We", [D, H], F32, kind="ExternalInput").ap()
    w_in["Wg"] = nc.dram_tensor("Wg", [D, H], F32, kind="ExternalInput").ap()
    w_in["Wgate"] = nc.dram_tensor("Wgate", [3 * D, 1], F32, kind="ExternalInput").ap()
    for name in ("ln1_g", "ln1_b", "ln2_g", "ln2_b"):
        w_in[name] = nc.dram_tensor(name, [D], F32, kind="ExternalInput").ap()
    out_d = nc.dram_tensor("out", [npc, D], F32, kind="ExternalOutput").ap()

    with tile.TileContext(nc) as tc:
        import contextlib

        ctx = contextlib.ExitStack()
        with ctx:
            dram = ctx.enter_context(tc.tile_pool(name="dram", bufs=1, space="DRAM"))
            k_own = dram.tile([npc, D], edt)
            eg1 = dram.tile([nblk, 128, M, 16], F32)

            consts = ctx.enter_context(tc.tile_pool(name="consts", bufs=1))

            # ---------- setup ----------
            iota_i = consts.tile([128, 128], I32)
            nc.gpsimd.iota(iota_i[:], pattern=[[1, 128]], base=0, channel_multiplier=0)
            iota_f = consts.tile([128, 128], F32)
            nc.vector.tensor_copy(out=iota_f[:], in_=iota_i[:])

            ident = consts.tile([128, 128], F32)
            make_identity(nc, ident[:])

            ones_row = consts.tile([1, 128], F32)
            nc.vector.memset(ones_row[:], 1.0)

            const2 = consts.tile([128, 2], F32)
            nc.vector.memset(const2[:, 0:1], 0.0)
            nc.vector.memset(const2[:, 1:2], EPS)
            nc.const_aps.aps[(F32, 0.0)] = const2[:, 0:1]
            nc.const_aps.aps[(F32, EPS)] = const2[:, 1:2]

            lnrow = consts.tile([1, 4 * D], F32)
            for i, name in enumerate(("ln1_g", "ln1_b", "ln2_g", "ln2_b")):
                nc.sync.dma_start(
                    out=lnrow[:, i * D : (i + 1) * D], in_=w_in[name][None, :]
                )
            lnb = consts.tile([128, 4 * D], F32)

            wqv32 = consts.tile([D, 2 * D], F32)
            nc.sync.dma_start(out=wqv32[:, 0:D], in_=w_in["Wq"][:])
            nc.sync.dma_start(out=wqv32[:, D : 2 * D], in_=w_in["Wv"][:])
            wqv16 = consts.tile([D, 2 * D], F16)
            nc.vector.tensor_copy(out=wqv16[:], in_=wqv32[:])
            wk_s = consts.tile([D, D], F32)
            nc.sync.dma_start(out=wk_s[:], in_=w_in["Wk"][:])
            weg32 = consts.tile([D, 2 * H], F32)
            nc.sync.dma_start(out=weg32[:, 0:H], in_=w_in["We"][:])
            nc.sync.dma_start(out=weg32[:, H : 2 * H], in_=w_in["Wg"][:])
            weg = consts.tile([D, 2 * H], F16)
            nc.vector.tensor_copy(out=weg[:], in_=weg32[:])
            w1_s = consts.tile([D, D], F32)
            nc.sync.dma_start(out=w1_s[:], in_=w_in["W1"][:])
            w2_s = consts.tile([D, D], F32)
            nc.sync.dma_start(out=w2_s[:], in_=w_in["W2"][:])

            # gate vector folding: gate_pre = agg@(Wo@A) + feat@(Wskip@B)
            #   A = Wgate[0:D] + Wgate[2D:3D], B = Wgate[D:2D] - Wgate[2D:3D]
            wg3 = consts.tile([128, 3], F32)
            nc.sync.dma_start(
                out=wg3[:], in_=w_in["Wgate"].rearrange("(t p) c -> p (t c)", p=128)
            )
            ab = consts.tile([128, 2], F32)
            nc.vector.tensor_add(out=ab[:, 0:1], in0=wg3[:, 0:1], in1=wg3[:, 2:3])
            nc.vector.tensor_sub(out=ab[:, 1:2], in0=wg3[:, 1:2], in1=wg3[:, 2:3])

            wot_s = consts.tile([D, D], F32)
            nc.sync.dma_start(out=wot_s[:], in_=w_in["WoT"][:])
            wskipt_s = consts.tile([D, D], F32)
            nc.sync.dma_start(out=wskipt_s[:], in_=w_in["WskipT"][:])

            rhs_o = consts.tile([D, D + 1], F32)
            nc.sync.dma_start(out=rhs_o[:, 0:D], in_=w_in["Wo"][:])
            rhs_s = consts.tile([D, D + 1], F32)
            nc.sync.dma_start(out=rhs_s[:, 1 : D + 1], in_=w_in["Wskip"][:])

            with tc.tile_pool(name="psum_setup", bufs=1, space="PSUM") as pss:
                ps_ln = pss.tile([128, 4 * D], F32, tag="ln")
                nc.tensor.matmul(
                    out=ps_ln[:], lhsT=ones_row[:], rhs=lnrow[:], start=True, stop=True
                )
                nc.vector.tensor_copy(out=lnb[:], in_=ps_ln[:])

                ps_c = pss.tile([128, 2], F32, tag="c")
                nc.tensor.matmul(
                    out=ps_c[:, 0:1], lhsT=wot_s[:], rhs=ab[:, 0:1],
                    start=True, stop=True,
                )
                nc.tensor.matmul(
                    out=ps_c[:, 1:2], lhsT=wskipt_s[:], rhs=ab[:, 1:2],
                    start=True, stop=True,
                )
                nc.vector.tensor_copy(out=rhs_o[:, D : D + 1], in_=ps_c[:, 0:1])
                nc.vector.tensor_copy(out=rhs_s[:, 0:1], in_=ps_c[:, 1:2])

            featT = consts.tile([D, npc], F32)
            nc.sync.dma_start(out=featT[:], in_=featT_d[:])

            # ---------- stage 1: k for own nodes (dst-indexed, core-local) ----------
            with tc.tile_pool(name="qkv_sb", bufs=2) as qsb, \
                 tc.tile_pool(name="qkv_ps", bufs=1, space="PSUM") as qps:
                for i0 in range(0, npc, 128):
                    ni = min(128, npc - i0)
                    ps_qkv = qps.tile([128, D], F32, tag="qkv")
                    nc.tensor.matmul(
                        out=ps_qkv[:ni], lhsT=featT[:, i0 : i0 + ni], rhs=wk_s[:],
                        start=True, stop=True,
                    )
                    qvk = qsb.tile([128, D], edt, tag="qvk")
                    nc.vector.tensor_copy(out=qvk[:ni], in_=ps_qkv[:ni])
                    nc.scalar.dma_start(
                        out=k_own[i0 : i0 + ni, :], in_=qvk[:ni]
                    )

            # ---------- stage 2: per-edge e_bias/gates ----------
            # eg1[b, p, j, 0:8]  = e_bias (raw)
            # eg1[b, p, j, 8:16] = sigmoid(edge_feat @ Wg)
            with tc.tile_pool(name="eg_sb", bufs=2) as esb, \
                 tc.tile_pool(name="eg_ps", bufs=1, space="PSUM") as eps:
                for b in range(nblk):
                    ef32_t = esb.tile([128, CAP], F32, tag="ef32")
                    nc.sync.dma_start(
                        out=ef32_t[:], in_=eft_d[:, b * CAP : (b + 1) * CAP]
                    )
                    ef_t = esb.tile([128, CAP], F16, tag="ef")
                    nc.scalar.activation(out=ef_t[:], in_=ef32_t[:], func=AF.Copy)
                    ps_eg = eps.tile([128, M, 2 * H], F32, tag="eg")
                    for j in range(M):
                        nc.tensor.matmul(
                            out=ps_eg[:, j, :],
                            lhsT=ef_t[:, j * 128 : (j + 1) * 128],
                            rhs=weg[:],
                            start=True, stop=True,
                        )
                    eg_t = esb.tile([128, M, 2 * H], F32, tag="egs")
                    nc.scalar.activation(
                        out=eg_t[:, :, 0:H], in_=ps_eg[:, :, 0:H], func=AF.Copy
                    )
                    # sigmoid(z) = 1/(1+exp(-z)) -- keeps ACT on the exp/ln set
                    sg_t = esb.tile([128, M, H], F32, tag="sg")
                    nc.scalar.activation(
                        out=sg_t[:], in_=ps_eg[:, :, H : 2 * H],
                        func=AF.Exp, scale=-1.0,
                    )
                    nc.vector.tensor_scalar_add(out=sg_t[:], in0=sg_t[:], scalar1=1.0)
                    nc.vector.reciprocal(out=eg_t[:, :, H : 2 * H], in_=sg_t[:])
                    nc.scalar.dma_start(out=eg1[b], in_=eg_t[:])

            # ---------- stage 3: edge aggregation + node epilogue ----------
            sb2 = ctx.enter_context(tc.tile_pool(name="p2_sb", bufs=2))
            sb2a = ctx.enter_context(tc.tile_pool(name="p2_sba", bufs=2))
            epi = ctx.enter_context(tc.tile_pool(name="epi_sb", bufs=2))
            ps_aggp = ctx.enter_context(tc.tile_pool(name="ps_agg", bufs=2, space="PSUM"))
            ps_rsp = ctx.enter_context(tc.tile_pool(name="ps_rs", bufs=1, space="PSUM"))
            ps_kexp = ctx.enter_context(tc.tile_pool(name="ps_kexp", bufs=2, space="PSUM"))
            ps_qvp = ctx.enter_context(tc.tile_pool(name="ps_qv", bufs=2, space="PSUM"))

            def layer_norm(x_t, g_col, b_col, out_t, nb):
                """LN over free dim (D) with per-feature affine from lnb."""
                nm = epi.tile([blk, 1], F32, tag="ln_nm")
                nc.vector.tensor_reduce(
                    out=nm[:nb], in_=x_t[:nb], axis=mybir.AxisListType.X,
                    op=OP.add, negate=True,
                )
                nm2 = epi.tile([blk, 1], F32, tag="ln_nm2")
                nc.scalar.activation(
                    out=nm2[:nb], in_=nm[:nb], func=AF.Copy, scale=1.0 / D
                )
                xc = epi.tile([blk, D], F32, tag="ln_xc")
                nc.scalar.activation(
                    out=xc[:nb], in_=x_t[:nb], func=AF.Identity, bias=nm2[:nb, 0:1]
                )
                sqd = epi.tile([blk, D], F32, tag="ln_sqd")
                v2 = epi.tile([blk, 1], F32, tag="ln_v2")
                nc.scalar.activation(
                    out=sqd[:nb], in_=xc[:nb], func=AF.Square, accum_out=v2[:nb]
                )
                sd = epi.tile([blk, 1], F32, tag="ln_sd")
                nc.scalar.activation(
                    out=sd[:nb], in_=v2[:nb], func=AF.Ln, scale=1.0 / D, bias=EPS
                )
                rstd = epi.tile([blk, 1], F32, tag="ln_rstd")
                nc.scalar.activation(
                    out=rstd[:nb], in_=sd[:nb], func=AF.Exp, scale=-0.5
                )
                nc.vector.scalar_tensor_tensor(
                    out=out_t[:nb], in0=xc[:nb], scalar=rstd[:nb, 0:1],
                    in1=lnb[:nb, g_col * D : (g_col + 1) * D],
                    op0=OP.mult, op1=OP.mult,
                )
                nc.vector.tensor_add(
                    out=out_t[:nb], in0=out_t[:nb],
                    in1=lnb[:nb, b_col * D : (b_col + 1) * D],
                )

            for b in range(nblk):
                # --- loads + per-edge q|v compute (no gathers) ---
                dloc_t = sb2.tile([128, M], I32, tag="dloc")
                nc.sync.dma_start(out=dloc_t[:], in_=dloc_d[b])
                fs32_t = sb2.tile([128, CAP], F32, tag="fs32")
                nc.sync.dma_start(
                    out=fs32_t[:], in_=fslotT_d[:, b * CAP : (b + 1) * CAP]
                )
                fs16_t = sb2.tile([128, CAP], F16, tag="fs16")
                nc.gpsimd.tensor_copy(out=fs16_t[:], in_=fs32_t[:])

                # k[dst] expansion on PE: k_tile = ohT.T @ k_block per tile.
                # K=125 contraction drops pad edges (dstloc=125..127) to zero.
                ohT_t = sb2.tile([128, M, 128], F8, tag="oht")
                nc.sync.dma_start(out=ohT_t[:], in_=ohT_d[b])
                k_blk = sb2.tile([blk, D], edt, tag="kblk")
                nc.sync.dma_start(out=k_blk[:], in_=k_own[b * blk : (b + 1) * blk, :])
                k_grps = []
                for g0 in range(0, M, 4):
                    ng = min(4, M - g0)
                    ps_k = ps_kexp.tile([128, 4, D], F32, tag="kexp")
                    for jj in range(ng):
                        nc.tensor.matmul(
                            out=ps_k[:, jj, :],
                            lhsT=ohT_t[:blk, g0 + jj, :],
                            rhs=k_blk[:],
                            start=True, stop=True,
                        )
                    k_s = sb2.tile([128, 4, D], F32, tag=f"ks{g0 % 8}")
                    nc.scalar.activation(
                        out=k_s[:, 0:ng, :], in_=ps_k[:, 0:ng, :], func=AF.Copy
                    )
                    k_grps.append((g0, ng, k_s))

                # q|v per edge: qv = fslot @ [Wq|Wv]  (fp16, psum groups of 2
                # tiles, drained to SBUF immediately by the idle gpsimd)
                qv_secs = []
                for gi, g0 in enumerate(range(0, M, 2)):
                    ng = min(2, M - g0)
                    ps_qv = ps_qvp.tile([128, 2, 2 * D], F32, tag="qv")
                    for jj in range(ng):
                        nc.tensor.matmul(
                            out=ps_qv[:, jj, :],
                            lhsT=fs16_t[:, (g0 + jj) * 128 : (g0 + jj + 1) * 128],
                            rhs=wqv16[:],
                            start=True, stop=True,
                        )
                    qv_sb = sb2.tile([128, 2, 2 * D], edt, tag=f"qvs{gi}")
                    nc.scalar.activation(
                        out=qv_sb[:, 0:ng, :], in_=ps_qv[:, 0:ng, :], func=AF.Copy
                    )
                    qv_secs.append((g0, ng, qv_sb))

                eg_t = sb2.tile([128, M, 2 * H], F32, tag="eg2")
                nc.sync.dma_start(out=eg_t[:], in_=eg1[b])

                # --- per-edge math ---
                dl_f = dloc_t[:].bitcast(F32)
                oh = sb2a.tile([128, M, 128], F32, tag="oh")
                nc.vector.tensor_tensor(
                    out=oh[:],
                    in0=iota_f[:][:, None, :].to_broadcast([128, M, 128]),
                    in1=dl_f[:, :, None].to_broadcast([128, M, 128]),
                    op=OP.is_equal,
                )
                qk = sb2a.tile([128, M, D], F32, tag="qk")
                for off, ms, qv_s in qv_secs:
                    for g0, ng, k_s in k_grps:
                        a0 = max(off, g0)
                        a1 = min(off + ms, g0 + ng)
                        if a0 >= a1:
                            continue
                        nc.vector.tensor_mul(
                            out=qk[:, a0:a1, :],
                            in0=qv_s[:, a0 - off : a1 - off, 0:D],
                            in1=k_s[:, a0 - g0 : a1 - g0, :],
                        )
                a_t = sb2a.tile([128, M * H], F32, tag="a")
                nc.vector.tensor_reduce(
                    out=a_t[:],
                    in_=qk[:].rearrange("p m (h x) -> p (m h) x", x=HD),
                    axis=mybir.AxisListType.X,
                    op=OP.add,
                )
                w_t = sb2a.tile([128, M * H], F32, tag="w")
                nc.vector.tensor_scalar(
                    out=w_t[:], in0=a_t[:], scalar1=5.0, scalar2=-5.0,
                    op0=OP.min, op1=OP.max,
                )
                # logits = (clip(a) + e_bias) * 4 ; x = exp(logits)
                nc.vector.tensor_add(
                    out=w_t[:].rearrange("p (m h) -> p m h", h=H),
                    in0=w_t[:].rearrange("p (m h) -> p m h", h=H),
                    in1=eg_t[:, :, 0:H],
                )
                x_t = sb2a.tile([128, M * H], F32, tag="x")
                nc.scalar.activation(out=x_t[:], in_=w_t[:], func=AF.Exp, scale=4.0)

                pu = sb2a.tile([128, M, H + D], F32, tag="pu")
                nc.vector.tensor_copy(
                    out=pu[:, :, 0:H],
                    in_=x_t[:].rearrange("p (m h) -> p m h", h=H),
                )
                pg_t = sb2a.tile([128, M, H], F32, tag="pg")
                nc.vector.tensor_mul(
                    out=pg_t[:],
                    in0=x_t[:].rearrange("p (m h) -> p m h", h=H),
                    in1=eg_t[:, :, H : 2 * H],
                )
                for off, ms, qv_s in qv_secs:
                    nc.vector.tensor_mul(
                        out=pu[:, off : off + ms, H : H + D].rearrange(
                            "p m (h x) -> p m h x", x=HD
                        ),
                        in0=qv_s[:, 0:ms, D : 2 * D].rearrange(
                            "p m (h x) -> p m h x", x=HD
                        ),
                        in1=pg_t[:, off : off + ms, :, None].to_broadcast(
                            [128, ms, H, HD]
                        ),
                    )

                ps_agg = ps_aggp.tile([128, H + D], F32, tag="agg")
                for j in range(M):
                    nc.tensor.matmul(
                        out=ps_agg[:],
                        lhsT=oh[:, j, :],
                        rhs=pu[:, j, :],
                        start=(j == 0),
                        stop=(j == M - 1),
                    )

                # --- node epilogue for this block ---
                nb = blk
                dsafe = epi.tile([blk, H], F32, tag="ds")
                nc.vector.tensor_scalar_max(
                    out=dsafe[:nb], in0=ps_agg[:nb, 0:H], scalar1=1e-30
                )
                dinv = epi.tile([blk, H], F32, tag="dinv")
                nc.vector.reciprocal(out=dinv[:nb], in_=dsafe[:nb])
                agg_s = epi.tile([blk, D], F32, tag="aggs")
                nc.vector.tensor_mul(
                    out=agg_s[:nb].rearrange("p (h x) -> p h x", x=HD),
                    in0=ps_agg[:nb, H : H + D].rearrange("p (h x) -> p h x", x=HD),
                    in1=dinv[:nb, :, None].to_broadcast([nb, H, HD]),
                )

                ps_tr = ps_aggp.tile([128, 3, blk], F32, tag="agg")
                nc.tensor.transpose(
                    out=ps_tr[:, 0, :], in_=agg_s[:nb], identity=ident[:nb, :nb]
                )
                aggT = epi.tile([D, blk], F32, tag="aggT")
                nc.scalar.activation(out=aggT[:], in_=ps_tr[:, 0, :], func=AF.Copy)

                # rsf bank layout: [0:128) rst -> later ffn2; 128 gp (both
                # matmuls accumulate via has_written bits); [129:257) skip ->
                # later ffn1.
                rsf = ps_rsp.tile([blk, 2 * D + 2], F32, tag="rs")
                nc.tensor.matmul(
                    out=rsf[:nb, 0 : D + 1], lhsT=aggT[:, :nb], rhs=rhs_o[:],
                    start=True, stop=True,
                )
                nc.tensor.matmul(
                    out=rsf[:nb, D + 1 : 2 * D + 2],
                    lhsT=featT[:, b * blk : b * blk + nb],
                    rhs=rhs_s[:],
                    start=True, stop=True,
                )
                sk_s = epi.tile([blk, D + 1], F32, tag="sk")
                nc.scalar.activation(
                    out=sk_s[:nb], in_=rsf[:nb, D + 1 : 2 * D + 2], func=AF.Copy
                )
                gp = epi.tile([blk, 1], F32, tag="gp")
                nc.vector.tensor_add(
                    out=gp[:nb], in0=rsf[:nb, D : D + 1], in1=sk_s[:nb, 0:1]
                )
                g_s = epi.tile([blk, 1], F32, tag="g")
                nc.scalar.activation(
                    out=g_s[:nb], in_=gp[:nb], func=AF.Exp, scale=-1.0
                )
                nc.vector.tensor_scalar_add(out=g_s[:nb], in0=g_s[:nb], scalar1=1.0)
                nc.vector.reciprocal(out=g_s[:nb], in_=g_s[:nb])
                diff = epi.tile([blk, D], F32, tag="diff")
                nc.vector.tensor_sub(
                    out=diff[:nb], in0=rsf[:nb, 0:D], in1=sk_s[:nb, 1 : D + 1]
                )
                mix = epi.tile([blk, D], F32, tag="mix")
                nc.vector.scalar_tensor_tensor(
                    out=mix[:nb], in0=diff[:nb], scalar=g_s[:nb, 0:1],
                    in1=sk_s[:nb, 1 : D + 1],
                    op0=OP.mult, op1=OP.add,
                )

                h_t = epi.tile([blk, D], F32, tag="h")
                layer_norm(mix, 0, 1, h_t, nb)
                l2 = epi.tile([blk, D], F32, tag="l2")
                layer_norm(h_t, 2, 3, l2, nb)

                nc.tensor.transpose(
                    out=ps_tr[:, 1, :], in_=l2[:nb], identity=ident[:nb, :nb]
                )
                l2T = epi.tile([D, blk], F32, tag="l2T")
                nc.scalar.activation(out=l2T[:], in_=ps_tr[:, 1, :], func=AF.Copy)
                nc.tensor.matmul(
                    out=rsf[:nb, D + 2 : 2 * D + 2], lhsT=l2T[:, :nb], rhs=w1_s[:],
                    start=True, stop=True,
                )
                r_t = epi.tile([blk, D], F32, tag="r")
                nc.scalar.activation(
                    out=r_t[:nb], in_=rsf[:nb, D + 2 : 2 * D + 2], func=AF.Relu
                )
                nc.tensor.transpose(
                    out=ps_tr[:, 2, :], in_=r_t[:nb], identity=ident[:nb, :nb]
                )
                rT = epi.tile([D, blk], F32, tag="rT")
                nc.scalar.activation(out=rT[:], in_=ps_tr[:, 2, :], func=AF.Copy)
                nc.tensor.matmul(
                    out=rsf[:nb, 0:D], lhsT=rT[:, :nb], rhs=w2_s[:],
                    start=True, stop=True,
                )
                outb = epi.tile([blk, D], F32, tag="outb")
                nc.vector.tensor_add(
                    out=outb[:nb], in0=h_t[:nb], in1=rsf[:nb, 0:D]
                )
                nc.scalar.dma_start(
                    out=out_d[b * blk : b * blk + nb, :], in_=outb[:nb]
                )

    nc.compile()
    return nc


def compute_layout(inputs, base):
    """Decide the data-dependent static block capacity M (tiles per block)."""
    cores, npc, nblk, blk = base["cores"], base["npc"], base["nblk"], base["blk"]
    nblk_g = cores * nblk

    src = np.asarray(inputs["src"]).astype(np.int64)
    dst = np.asarray(inputs["dst"]).astype(np.int64)
    gb_all = dst // blk
    order = np.lexsort((src, gb_all))  # by block, then src
    ds = dst[order]
    ss = src[order]
    gb = gb_all[order]

    counts = np.bincount(gb, minlength=nblk_g)
    M = max(2, int(np.ceil(counts.max() / 128)))

    starts = np.zeros(nblk_g + 1, dtype=np.int64)
    np.cumsum(counts, out=starts[1:])
    pos = np.arange(len(ds)) - starts[gb]
    slot = gb * (M * 128) + pos

    layout = dict(order=order, ds=ds, ss=ss, gb=gb, slot=slot)
    cfg = dict(base, M=M, HM=0)
    return cfg, layout


def _wrap16(arr):
    """[nblk, C] -> [nblk, 128, C//16] interleaved 16-wrap, replicated x8."""
    nblk, C = arr.shape
    base = arr.reshape(nblk, C // 16, 16).transpose(0, 2, 1)  # [nblk, 16, C//16]
    return np.ascontiguousarray(
        np.tile(base, (1, 8, 1)).astype(np.int16)
    )


def shard_inputs(inputs, cfg, layout):
    """Host-side layout only (sort/pad/transpose/index, no arithmetic)."""
    cores = cfg["cores"]
    npc = cfg["npc"]
    nblk = cfg["nblk"]
    blk = cfg["blk"]
    M = cfg["M"]
    CAP = M * 128
    nblk_g = cores * nblk

    ds, ss, slot = layout["ds"], layout["ss"], layout["slot"]
    gb = layout["gb"]
    edge_feat = np.asarray(inputs["edge_feat"])
    feat = np.asarray(inputs["feat"])

    total = nblk_g * CAP
    dstloc = np.full(total, float(blk), dtype=np.float32)
    dstloc[slot] = (ds - gb * blk).astype(np.float32)

    ef_pad = np.zeros((total, D), dtype=np.float32)
    ef_pad[slot] = edge_feat[layout["order"]]
    fs_pad = np.zeros((total, D), dtype=np.float32)
    fs_pad[slot] = feat[ss]

    # transposed one-hot for the PE k-expansion: ohT[b, n, j, p] = 1 iff
    # dst_local(edge at slot j*128+p of block b) == n  (pads land on n=blk)
    f8 = mybir.dt.np(F8)
    ohT = np.zeros(nblk_g * 128 * CAP, dtype=f8)
    n_l = dstloc.astype(np.int64)
    sb_ = np.arange(total) % CAP
    gb_s = np.arange(total) // CAP
    oh_idx = ((gb_s * 128 + n_l) * (CAP // 128) + sb_ // 128) * 128 + sb_ % 128
    ohT[oh_idx] = 1.0
    ohT = ohT.reshape(nblk_g, 128, CAP // 128, 128)

    per_core = nblk * CAP
    in_maps = []
    for c_i in range(cores):
        bsl = slice(c_i * nblk, (c_i + 1) * nblk)
        sl = slice(c_i * per_core, (c_i + 1) * per_core)
        dloc = np.ascontiguousarray(
            dstloc[sl].reshape(nblk, M, 128).transpose(0, 2, 1)
        ).view(np.int32)

        m = {
            "eft": np.ascontiguousarray(ef_pad[sl].T),
            "fslotT": np.ascontiguousarray(fs_pad[sl].T),
            "dloc": dloc,
            "ohT": np.ascontiguousarray(ohT[bsl]),
            "featT": np.ascontiguousarray(
                np.asarray(inputs["feat"])[c_i * npc : (c_i + 1) * npc].T
            ),
            "WoT": np.ascontiguousarray(np.asarray(inputs["Wo"]).T),
            "WskipT": np.ascontiguousarray(np.asarray(inputs["Wskip"]).T),
        }
        for name in ("Wq", "Wk", "Wv", "Wo", "Wskip", "W1", "W2", "We", "Wg",
                     "Wgate", "ln1_g", "ln1_b", "ln2_g", "ln2_b"):
            m[name] = np.ascontiguousarray(np.asarray(inputs[name]))
        in_maps.append(m)
    return in_maps


_cache = {}


def _get_program(cfg):
    key = (cfg["cores"], cfg["n_nodes"], cfg["M"], cfg["HM"], EDGE_DT)
    if key not in _cache:
        _cache[key] = build_program(cfg)
    return _cache[key]


def full_base():
    return dict(cores=CORES, n_nodes=N_NODES, npc=NPC, nblk=NBLK, blk=BLK)


def _ensure_ntff_hook():
    """The agent image's antenv lacks axon_hooks; synthesize it from the
    boot module's ctypes NTFF profiler so trace=True can capture timings."""
    import types

    if "antenv.axon_hooks" in sys.modules:
        return
    try:
        sys.path.insert(0, "/root/.axon_site")
        from trn_agent_boot.trn_boot import _ntff_profile_via_ctypes

        hook = _ntff_profile_via_ctypes("/opt/axon/libaxon_pjrt.so")
        mod = types.ModuleType("antenv.axon_hooks")
        mod.get_axon_ntff_profile_hook = lambda: hook
        mod.set_axon_ntff_profile_hook = lambda h: None
        sys.modules["antenv.axon_hooks"] = mod
    except Exception as e:  # degrade to untimed run
        print(f"ntff hook setup failed: {e}")


def run(inputs, trace=False, tmpdir=None, trace_cores=None):
    if trace:
        _ensure_ntff_hook()
    cfg, layout = compute_layout(inputs, full_base())
    nc = _get_program(cfg)
    in_maps = shard_inputs(inputs, cfg, layout)
    res = bass_utils.run_bass_kernel_spmd(
        nc,
        in_maps,
        core_ids=list(range(cfg["cores"])),
        trace=trace,
        tmpdir=tmpdir,
        trace_cores=trace_cores,
    )
    out = np.concatenate([res.results[c]["out"] for c in range(cfg["cores"])], axis=0)
    return out, res


def kernel(**inputs):
    out, _ = run(inputs)
    return out



# revision 26
# speedup vs baseline: 1.7949x; 1.7949x over previous
"""Trainium2 Bass kernel for GAT-style GNN message passing (edge softmax).

Contract: kernel(**inputs) takes FULL unsharded numpy inputs, distributes
across 8 NeuronCores internally, returns FULL output.

Sharding: edges sorted by dst and partitioned by dst range (6250 nodes per
core) -> every per-destination segment reduction is core-local. Node
features/weights replicated.

v2 design (single fused pass over 50 dst-blocks of 125 nodes):
- host ships fslotT/edge_featT in f16 (halves the two dominant DMA streams,
  removes on-chip casts) and BOTH one-hot orientations (oh + ohT) as f8.
- per-block, per 4-tile group: q/k/eg matmuls into PSUM, qk product read
  directly from PSUM (no PSUM->SBUF drains of q/k/v at all).
- head-dim dot via f16 2x tree-reduce instead of tensor_reduce.
- exp values kept in bf16 (same exponent range as f32) so the aggregation
  matmul runs at 1 cy/row instead of fp32's 4.
- second PE pass recomputes v per group; pu=v*pg multiplied straight from
  PSUM; agg matmul lhsT is the host-shipped f8 one-hot.
- epilogue (gated residual + 2xLN + FFN) as baseline but f16 weights.
"""

import os
import sys

sys.path.insert(0, "/opt/trn_rl_repo")

import numpy as np

import concourse.bass as bass
import concourse.mybir as mybir
import concourse.tile as tile
from concourse import bacc
from concourse import bass_utils
from concourse.masks import make_identity

F32 = mybir.dt.float32
F16 = mybir.dt.float16
BF16 = mybir.dt.bfloat16
F8 = mybir.dt.float8e4
I32 = mybir.dt.int32
AF = mybir.ActivationFunctionType
OP = mybir.AluOpType

D = 128
H = 8
HD = 16
EPS = 1e-5

N_NODES = 50000
N_EDGES = 800000
CORES = 8
NPC = N_NODES // CORES      # nodes per core = 6250
BLK = 125                   # dst nodes per block
NBLK = NPC // BLK           # 50 blocks per core
G = 4                       # edge tiles per PSUM group

# gpsimd cannot access PSUM; it gets the SBUF-only elementwise work
TREE_GPS = int(os.environ.get("KERNEL_TREE_GPS", "0"))
GPS_PW = int(os.environ.get("KERNEL_GPS_PW", "0"))


def _force_act_set():
    """Pin every ACTIVATE to the natural_log_exp_and_others table so the
    kernel pays one ACT_TABLE_LOAD instead of hundreds."""
    from concourse import hw_specs

    if getattr(bacc, "_act_set_forced", False):
        return
    real = hw_specs.get_activation_tables

    def patched(arch):
        t = dict(real(arch))
        keep = "natural_log_exp_and_others"
        return {name: (fns if name == keep else set()) for name, fns in t.items()}

    bacc.get_activation_tables = patched
    bacc._act_set_forced = True


def build_program(cfg):
    _force_act_set()
    cores = cfg["cores"]
    npc = cfg["npc"]
    nblk = cfg["nblk"]
    blk = cfg["blk"]
    M = cfg["M"]
    CAP = M * 128
    EPC = nblk * CAP

    nc = bacc.Bacc(
        "TRN2", target_bir_lowering=False, debug=False, num_devices=cores
    )

    # ---- I/O ----
    fsT_d = nc.dram_tensor("fsT", [D, EPC], F16, kind="ExternalInput").ap()
    efT_d = nc.dram_tensor("efT", [D, EPC], F16, kind="ExternalInput").ap()
    ohT_d = nc.dram_tensor("ohT", [nblk, 128, M, 128], F8, kind="ExternalInput").ap()
    oh_d = nc.dram_tensor("oh", [nblk, 128, M, 128], F8, kind="ExternalInput").ap()
    featT_d = nc.dram_tensor("featT", [D, npc], F16, kind="ExternalInput").ap()
    w_in = {}
    for name in ("Wq16", "Wk16", "Wv16", "Wo16", "Wskip16", "W116", "W216"):
        w_in[name] = nc.dram_tensor(name, [D, D], F16, kind="ExternalInput").ap()
    for name in ("WoT", "WskipT"):
        w_in[name] = nc.dram_tensor(name, [D, D], F32, kind="ExternalInput").ap()
    w_in["Weg16"] = nc.dram_tensor("Weg16", [D, 2 * H], F16, kind="ExternalInput").ap()
    w_in["Wgate"] = nc.dram_tensor("Wgate", [3 * D, 1], F32, kind="ExternalInput").ap()
    for name in ("ln1_g", "ln1_b", "ln2_g", "ln2_b"):
        w_in[name] = nc.dram_tensor(name, [D], F32, kind="ExternalInput").ap()
    out_d = nc.dram_tensor("out", [npc, D], F32, kind="ExternalOutput").ap()

    with tile.TileContext(nc) as tc:
        import contextlib

        ctx = contextlib.ExitStack()
        with ctx:
            consts = ctx.enter_context(tc.tile_pool(name="consts", bufs=1))

            # ---------- setup ----------
            ident = consts.tile([128, 128], F32)
            make_identity(nc, ident[:])

            ones_row = consts.tile([1, 128], F32)
            nc.vector.memset(ones_row[:], 1.0)

            const2 = consts.tile([128, 2], F32)
            nc.vector.memset(const2[:, 0:1], 0.0)
            nc.vector.memset(const2[:, 1:2], EPS)
            nc.const_aps.aps[(F32, 0.0)] = const2[:, 0:1]
            nc.const_aps.aps[(F32, EPS)] = const2[:, 1:2]

            lnrow = consts.tile([1, 4 * D], F32)
            for i, name in enumerate(("ln1_g", "ln1_b", "ln2_g", "ln2_b")):
                nc.sync.dma_start(
                    out=lnrow[:, i * D : (i + 1) * D], in_=w_in[name][None, :]
                )
            lnb = consts.tile([128, 4 * D], F32)

            wq16 = consts.tile([D, D], F16)
            nc.sync.dma_start(out=wq16[:], in_=w_in["Wq16"][:])
            wv16 = consts.tile([D, D], F16)
            nc.sync.dma_start(out=wv16[:], in_=w_in["Wv16"][:])
            wk16 = consts.tile([D, D], F16)
            nc.sync.dma_start(out=wk16[:], in_=w_in["Wk16"][:])
            weg16 = consts.tile([D, 2 * H], F16)
            nc.sync.dma_start(out=weg16[:], in_=w_in["Weg16"][:])
            w1_16 = consts.tile([D, D], F16)
            nc.sync.dma_start(out=w1_16[:], in_=w_in["W116"][:])
            w2_16 = consts.tile([D, D], F16)
            nc.sync.dma_start(out=w2_16[:], in_=w_in["W216"][:])

            # gate vector folding: gate_pre = agg@(Wo@A) + feat@(Wskip@B)
            wg3 = consts.tile([128, 3], F32)
            nc.sync.dma_start(
                out=wg3[:], in_=w_in["Wgate"].rearrange("(t p) c -> p (t c)", p=128)
            )
            ab = consts.tile([128, 2], F32)
            nc.vector.tensor_add(out=ab[:, 0:1], in0=wg3[:, 0:1], in1=wg3[:, 2:3])
            nc.vector.tensor_sub(out=ab[:, 1:2], in0=wg3[:, 1:2], in1=wg3[:, 2:3])

            wot_s = consts.tile([D, D], F32)
            nc.sync.dma_start(out=wot_s[:], in_=w_in["WoT"][:])
            wskipt_s = consts.tile([D, D], F32)
            nc.sync.dma_start(out=wskipt_s[:], in_=w_in["WskipT"][:])

            rhs_o = consts.tile([D, D + 1], F16)
            nc.sync.dma_start(out=rhs_o[:, 0:D], in_=w_in["Wo16"][:])
            rhs_s = consts.tile([D, D + 1], F16)
            nc.sync.dma_start(out=rhs_s[:, 1 : D + 1], in_=w_in["Wskip16"][:])

            featT = consts.tile([D, npc], F16)
            nc.sync.dma_start(out=featT[:], in_=featT_d[:])

            k_all = consts.tile([128, nblk, D], F16)

            with tc.tile_pool(name="psum_setup", bufs=1, space="PSUM") as pss:
                ps_ln = pss.tile([128, 4 * D], F32, tag="ln")
                nc.tensor.matmul(
                    out=ps_ln[:], lhsT=ones_row[:], rhs=lnrow[:], start=True, stop=True
                )
                nc.vector.tensor_copy(out=lnb[:], in_=ps_ln[:])

                ps_c = pss.tile([128, 2], F32, tag="c")
                nc.tensor.matmul(
                    out=ps_c[:, 0:1], lhsT=wot_s[:], rhs=ab[:, 0:1],
                    start=True, stop=True,
                )
                nc.tensor.matmul(
                    out=ps_c[:, 1:2], lhsT=wskipt_s[:], rhs=ab[:, 1:2],
                    start=True, stop=True,
                )
                nc.vector.tensor_copy(out=rhs_o[:, D : D + 1], in_=ps_c[:, 0:1])
                nc.vector.tensor_copy(out=rhs_s[:, 0:1], in_=ps_c[:, 1:2])

            # ---------- stage 1: k for own nodes, kept resident in SBUF ----------
            with tc.tile_pool(name="k_ps", bufs=2, space="PSUM") as kps:
                for b in range(nblk):
                    ps_kb = kps.tile([128, D], F32, tag="kb")
                    nc.tensor.matmul(
                        out=ps_kb[:blk],
                        lhsT=featT[:, b * blk : (b + 1) * blk],
                        rhs=wk16[:],
                        start=True, stop=True,
                    )
                    nc.scalar.activation(
                        out=k_all[:blk, b, :], in_=ps_kb[:blk], func=AF.Copy
                    )

            # ---------- main loop ----------
            sbA = ctx.enter_context(tc.tile_pool(name="sbA", bufs=2))
            sbB = ctx.enter_context(tc.tile_pool(name="sbB", bufs=2))
            epi = ctx.enter_context(tc.tile_pool(name="epi", bufs=2))
            # bank budget (8): q/v pool 3 + k 2 + agg 1 + eg 1 + epi 1.
            # the agg accumulation group must own its bank exclusively: a
            # start=True matmul clears has_written bits for its whole 2KB
            # zero region, which would turn pending accumulates into
            # overwrites (single start+stop matmuls are safe to co-locate).
            ps_qp = ctx.enter_context(tc.tile_pool(name="ps_q", bufs=3, space="PSUM"))
            ps_kp = ctx.enter_context(tc.tile_pool(name="ps_k", bufs=2, space="PSUM"))
            ps_aggp = ctx.enter_context(tc.tile_pool(name="ps_agg", bufs=1, space="PSUM"))
            ps_egp = ctx.enter_context(tc.tile_pool(name="ps_eg", bufs=1, space="PSUM"))
            ps_epip = ctx.enter_context(tc.tile_pool(name="ps_epi", bufs=1, space="PSUM"))

            def layer_norm(x_t, g_col, b_col, out_t, nb):
                nm = epi.tile([blk, 1], F32, tag="ln_nm")
                nc.vector.tensor_reduce(
                    out=nm[:nb], in_=x_t[:nb], axis=mybir.AxisListType.X,
                    op=OP.add, negate=True,
                )
                nm2 = epi.tile([blk, 1], F32, tag="ln_nm2")
                nc.scalar.activation(
                    out=nm2[:nb], in_=nm[:nb], func=AF.Copy, scale=1.0 / D
                )
                xc = epi.tile([blk, D], F32, tag="ln_xc")
                nc.scalar.activation(
                    out=xc[:nb], in_=x_t[:nb], func=AF.Identity, bias=nm2[:nb, 0:1]
                )
                sqd = epi.tile([blk, D], F32, tag="ln_sqd")
                v2 = epi.tile([blk, 1], F32, tag="ln_v2")
                nc.scalar.activation(
                    out=sqd[:nb], in_=xc[:nb], func=AF.Square, accum_out=v2[:nb]
                )
                sd = epi.tile([blk, 1], F32, tag="ln_sd")
                nc.scalar.activation(
                    out=sd[:nb], in_=v2[:nb], func=AF.Ln, scale=1.0 / D, bias=EPS
                )
                rstd = epi.tile([blk, 1], F32, tag="ln_rstd")
                nc.scalar.activation(
                    out=rstd[:nb], in_=sd[:nb], func=AF.Exp, scale=-0.5
                )
                nc.vector.scalar_tensor_tensor(
                    out=out_t[:nb], in0=xc[:nb], scalar=rstd[:nb, 0:1],
                    in1=lnb[:nb, g_col * D : (g_col + 1) * D],
                    op0=OP.mult, op1=OP.mult,
                )
                nc.vector.tensor_add(
                    out=out_t[:nb], in0=out_t[:nb],
                    in1=lnb[:nb, b_col * D : (b_col + 1) * D],
                )

            ngroups = (M + G - 1) // G

            for b in range(nblk):
                # --- loads ---
                fs16 = sbA.tile([128, CAP], F16, tag="fs")
                nc.sync.dma_start(out=fs16[:], in_=fsT_d[:, b * CAP : (b + 1) * CAP])
                ef16 = sbA.tile([128, CAP], F16, tag="ef")
                nc.scalar.dma_start(out=ef16[:], in_=efT_d[:, b * CAP : (b + 1) * CAP])
                ohT_t = sbA.tile([128, M, 128], F8, tag="ohT")
                nc.sync.dma_start(out=ohT_t[:], in_=ohT_d[b])
                oh_t = sbA.tile([128, M, 128], F8, tag="oh")
                nc.scalar.dma_start(out=oh_t[:], in_=oh_d[b])

                ps_agg = ps_aggp.tile([128, H + D], F32, tag="agg")
                ps_eg = ps_egp.tile([128, M, 2 * H], F32, tag="eg")

                qk_t = sbB.tile([128, M, 128], F16, tag="qk")
                k16 = sbB.tile([128, M, 128], F16, tag="k16")

                # --- pass A: q, k, eg matmuls + qk product per group ---
                for g0 in range(0, M, G):
                    ng = min(G, M - g0)
                    ps_q = ps_qp.tile([128, G, 128], F32, tag="q")
                    ps_k = ps_kp.tile([128, G, 128], F32, tag="k")
                    for jj in range(ng):
                        j = g0 + jj
                        nc.tensor.matmul(
                            out=ps_q[:, jj, :],
                            lhsT=fs16[:, j * 128 : (j + 1) * 128],
                            rhs=wq16[:],
                            start=True, stop=True,
                        )
                        nc.tensor.matmul(
                            out=ps_k[:, jj, :],
                            lhsT=ohT_t[:blk, j, :],
                            rhs=k_all[:blk, b, :],
                            start=True, stop=True,
                        )
                        nc.tensor.matmul(
                            out=ps_eg[:, j, :],
                            lhsT=ef16[:, j * 128 : (j + 1) * 128],
                            rhs=weg16[:],
                            start=True, stop=True,
                        )
                    # <=1 PSUM input per DVE op: drain k to SBUF f16 first
                    nc.scalar.activation(
                        out=k16[:, g0 : g0 + ng, :], in_=ps_k[:, 0:ng, :],
                        func=AF.Copy,
                    )
                    nc.vector.tensor_tensor(
                        out=qk_t[:, g0 : g0 + ng, :],
                        in0=ps_q[:, 0:ng, :],
                        in1=k16[:, g0 : g0 + ng, :],
                        op=OP.mult,
                    )

                # --- pass B: block-level pointwise ---
                # head-dim dot as a pairwise tree; level 1 (the big one) on
                # the otherwise-idle gpsimd, the rest on vector
                teng = nc.gpsimd if TREE_GPS else nc.vector
                qk3 = qk_t[:].rearrange("p m (h x) -> p (m h) x", x=HD)
                t1 = sbB.tile([128, M * H, 8], F16, tag="t1")
                teng.tensor_tensor(
                    out=t1[:], in0=qk3[:, :, 0:8], in1=qk3[:, :, 8:16], op=OP.add
                )
                t2 = sbB.tile([128, M * H, 4], F16, tag="t2")
                nc.vector.tensor_add(
                    out=t2[:], in0=t1[:, :, 0:4], in1=t1[:, :, 4:8]
                )
                t3 = sbB.tile([128, M * H, 2], F16, tag="t3")
                nc.vector.tensor_add(
                    out=t3[:], in0=t2[:, :, 0:2], in1=t2[:, :, 2:4]
                )
                a_t = sbB.tile([128, M * H], F16, tag="a")
                nc.vector.tensor_add(
                    out=a_t[:],
                    in0=t3[:].rearrange("p f two -> p (f two)")[:, 0::2],
                    in1=t3[:].rearrange("p f two -> p (f two)")[:, 1::2],
                )
                w_t = sbB.tile([128, M * H], F16, tag="w")
                peng = nc.gpsimd if GPS_PW else nc.vector
                peng.tensor_scalar(
                    out=w_t[:], in0=a_t[:], scalar1=5.0, scalar2=-5.0,
                    op0=OP.min, op1=OP.max,
                )
                wv = w_t[:].rearrange("p (m h) -> p m h", h=H)
                nc.vector.tensor_add(out=wv, in0=wv, in1=ps_eg[:, :, 0:H])

                pu_t = sbB.tile([128, M, H + D], BF16, tag="pu")
                nc.scalar.activation(
                    out=pu_t[:, :, 0:H], in_=wv, func=AF.Exp, scale=4.0
                )
                gex = sbB.tile([128, M, H], F32, tag="gex")
                nc.scalar.activation(
                    out=gex[:], in_=ps_eg[:, :, H : 2 * H], func=AF.Exp, scale=-1.0
                )
                peng.tensor_scalar_add(out=gex[:], in0=gex[:], scalar1=1.0)
                ginv = sbB.tile([128, M, H], F32, tag="ginv")
                nc.vector.reciprocal_approx_fast(
                    out=ginv[:].rearrange("p m h -> p (m h)"),
                    in_=gex[:].rearrange("p m h -> p (m h)"),
                )
                pg = sbB.tile([128, M, H], BF16, tag="pg")
                peng.tensor_tensor(
                    out=pg[:], in0=pu_t[:, :, 0:H], in1=ginv[:], op=OP.mult
                )

                # --- pass C: v matmuls, pu product, aggregation ---
                for g0 in range(0, M, G):
                    ng = min(G, M - g0)
                    ps_v = ps_qp.tile([128, G, 128], F32, tag="q")
                    for jj in range(ng):
                        j = g0 + jj
                        nc.tensor.matmul(
                            out=ps_v[:, jj, :],
                            lhsT=fs16[:, j * 128 : (j + 1) * 128],
                            rhs=wv16[:],
                            start=True, stop=True,
                        )
                    nc.vector.tensor_tensor(
                        out=pu_t[:, g0 : g0 + ng, H : H + D].rearrange(
                            "p m (h x) -> p m h x", x=HD
                        ),
                        in0=ps_v[:, 0:ng, :].rearrange("p m (h x) -> p m h x", x=HD),
                        in1=pg[:, g0 : g0 + ng, :, None].to_broadcast(
                            [128, ng, H, HD]
                        ),
                        op=OP.mult,
                    )
                    for jj in range(ng):
                        j = g0 + jj
                        nc.tensor.matmul(
                            out=ps_agg[:],
                            lhsT=oh_t[:, j, :],
                            rhs=pu_t[:, j, :],
                            start=(j == 0),
                            stop=(j == M - 1),
                        )

                # --- epilogue for this block ---
                nb = blk
                # ps_epi: [0:2D+2) rsf, [2D+2:2D+2+blk) transpose scratch
                ps_epi = ps_epip.tile([128, 2 * D + 2 + blk], F32, tag="epi")
                TR0 = 2 * D + 2

                dsafe = epi.tile([blk, H], F32, tag="ds")
                nc.vector.tensor_scalar_max(
                    out=dsafe[:nb], in0=ps_agg[:nb, 0:H], scalar1=1e-30
                )
                dinv = epi.tile([blk, H], F32, tag="dinv")
                nc.vector.reciprocal(out=dinv[:nb], in_=dsafe[:nb])
                agg_s = epi.tile([blk, D], F32, tag="aggs")
                nc.vector.tensor_mul(
                    out=agg_s[:nb].rearrange("p (h x) -> p h x", x=HD),
                    in0=ps_agg[:nb, H : H + D].rearrange("p (h x) -> p h x", x=HD),
                    in1=dinv[:nb, :, None].to_broadcast([nb, H, HD]),
                )

                nc.tensor.transpose(
                    out=ps_epi[:, TR0 : TR0 + blk], in_=agg_s[:nb],
                    identity=ident[:nb, :nb],
                )
                aggT = epi.tile([D, blk], F16, tag="aggT")
                nc.scalar.activation(
                    out=aggT[:], in_=ps_epi[:, TR0 : TR0 + blk], func=AF.Copy
                )

                nc.tensor.matmul(
                    out=ps_epi[:nb, 0 : D + 1], lhsT=aggT[:, :nb], rhs=rhs_o[:],
                    start=True, stop=True,
                )
                nc.tensor.matmul(
                    out=ps_epi[:nb, D + 1 : 2 * D + 2],
                    lhsT=featT[:, b * blk : b * blk + nb],
                    rhs=rhs_s[:],
                    start=True, stop=True,
                )
                sk_s = epi.tile([blk, D + 1], F32, tag="sk")
                nc.scalar.activation(
                    out=sk_s[:nb], in_=ps_epi[:nb, D + 1 : 2 * D + 2], func=AF.Copy
                )
                gp = epi.tile([blk, 1], F32, tag="gp")
                nc.vector.tensor_add(
                    out=gp[:nb], in0=ps_epi[:nb, D : D + 1], in1=sk_s[:nb, 0:1]
                )
                g_s = epi.tile([blk, 1], F32, tag="g")
                nc.scalar.activation(
                    out=g_s[:nb], in_=gp[:nb], func=AF.Exp, scale=-1.0
                )
                nc.vector.tensor_scalar_add(out=g_s[:nb], in0=g_s[:nb], scalar1=1.0)
                nc.vector.reciprocal(out=g_s[:nb], in_=g_s[:nb])
                diff = epi.tile([blk, D], F32, tag="diff")
                nc.vector.tensor_sub(
                    out=diff[:nb], in0=ps_epi[:nb, 0:D], in1=sk_s[:nb, 1 : D + 1]
                )
                mix = epi.tile([blk, D], F32, tag="mix")
                nc.vector.scalar_tensor_tensor(
                    out=mix[:nb], in0=diff[:nb], scalar=g_s[:nb, 0:1],
                    in1=sk_s[:nb, 1 : D + 1],
                    op0=OP.mult, op1=OP.add,
                )

                h_t = epi.tile([blk, D], F32, tag="h")
                layer_norm(mix, 0, 1, h_t, nb)
                l2 = epi.tile([blk, D], F32, tag="l2")
                layer_norm(h_t, 2, 3, l2, nb)

                nc.tensor.transpose(
                    out=ps_epi[:, TR0 : TR0 + blk], in_=l2[:nb],
                    identity=ident[:nb, :nb],
                )
                l2T = epi.tile([D, blk], F16, tag="l2T")
                nc.scalar.activation(
                    out=l2T[:], in_=ps_epi[:, TR0 : TR0 + blk], func=AF.Copy
                )
                nc.tensor.matmul(
                    out=ps_epi[:nb, D + 2 : 2 * D + 2], lhsT=l2T[:, :nb],
                    rhs=w1_16[:],
                    start=True, stop=True,
                )
                r_t = epi.tile([blk, D], F32, tag="r")
                nc.scalar.activation(
                    out=r_t[:nb], in_=ps_epi[:nb, D + 2 : 2 * D + 2], func=AF.Relu
                )
                nc.tensor.transpose(
                    out=ps_epi[:, TR0 : TR0 + blk], in_=r_t[:nb],
                    identity=ident[:nb, :nb],
                )
                rT = epi.tile([D, blk], F16, tag="rT")
                nc.scalar.activation(
                    out=rT[:], in_=ps_epi[:, TR0 : TR0 + blk], func=AF.Copy
                )
                nc.tensor.matmul(
                    out=ps_epi[:nb, 0:D], lhsT=rT[:, :nb], rhs=w2_16[:],
                    start=True, stop=True,
                )
                outb = epi.tile([blk, D], F32, tag="outb")
                nc.vector.tensor_add(
                    out=outb[:nb], in0=h_t[:nb], in1=ps_epi[:nb, 0:D]
                )
                nc.sync.dma_start(
                    out=out_d[b * blk : b * blk + nb, :], in_=outb[:nb]
                )

    nc.compile()
    return nc


def _balance_blocks(deg, nblk, blk):
    """LPT-pack nodes into nblk blocks of exactly blk nodes, equalizing the
    per-block edge load. Returns newid[orig_local] -> new local id."""
    import heapq

    npc = len(deg)
    order = np.argsort(-deg, kind="stable")
    cnt = np.zeros(nblk, dtype=np.int64)
    heap = [(0, b) for b in range(nblk)]
    heapq.heapify(heap)
    newid = np.empty(npc, dtype=np.int64)
    for n in order:
        while True:
            load, b = heapq.heappop(heap)
            if cnt[b] < blk:
                break
        newid[n] = b * blk + cnt[b]
        cnt[b] += 1
        if cnt[b] < blk:
            heapq.heappush(heap, (load + deg[n], b))
    return newid


def compute_layout(inputs, base):
    """Permute nodes within each core so per-block edge loads are balanced
    (lower static block capacity M), then lay edges out by dst block."""
    cores, npc, nblk, blk = base["cores"], base["npc"], base["nblk"], base["blk"]
    nblk_g = cores * nblk

    src = np.asarray(inputs["src"]).astype(np.int64)
    dst = np.asarray(inputs["dst"]).astype(np.int64)

    # per-core node permutation (new local id = block*blk + slot)
    newid = np.empty(cores * npc, dtype=np.int64)
    for c in range(cores):
        deg = np.bincount(dst[(dst >= c * npc) & (dst < (c + 1) * npc)] - c * npc,
                          minlength=npc)
        newid[c * npc : (c + 1) * npc] = c * npc + _balance_blocks(deg, nblk, blk)

    dstp = newid[dst]
    gb_all = dstp // blk
    order = np.lexsort((src, gb_all))
    ds = dstp[order]
    ss = src[order]
    gb = gb_all[order]

    counts = np.bincount(gb, minlength=nblk_g)
    M = max(2, int(np.ceil(counts.max() / 128)))

    starts = np.zeros(nblk_g + 1, dtype=np.int64)
    np.cumsum(counts, out=starts[1:])
    pos = np.arange(len(ds)) - starts[gb]
    slot = gb * (M * 128) + pos

    layout = dict(order=order, ds=ds, ss=ss, gb=gb, slot=slot, newid=newid)
    cfg = dict(base, M=M)
    return cfg, layout


def shard_inputs(inputs, cfg, layout):
    """Host-side layout only (sort/pad/transpose/index/cast, no arithmetic)."""
    cores = cfg["cores"]
    npc = cfg["npc"]
    nblk = cfg["nblk"]
    blk = cfg["blk"]
    M = cfg["M"]
    CAP = M * 128
    nblk_g = cores * nblk

    ds, ss, slot = layout["ds"], layout["ss"], layout["slot"]
    gb = layout["gb"]
    edge_feat = np.asarray(inputs["edge_feat"])
    feat = np.asarray(inputs["feat"])
    featp = np.empty_like(feat)
    featp[layout["newid"]] = feat

    total = nblk_g * CAP
    dstloc = np.full(total, blk, dtype=np.int64)
    dstloc[slot] = ds - gb * blk

    ef_pad = np.zeros((total, D), dtype=np.float16)
    ef_pad[slot] = edge_feat[layout["order"]]
    fs_pad = np.zeros((total, D), dtype=np.float16)
    fs_pad[slot] = feat[ss]

    f8 = mybir.dt.np(F8)
    sb_ = np.arange(total) % CAP
    gb_s = np.arange(total) // CAP
    # ohT[b, n, j, e] = 1 iff dst_local(slot j*128+e of block b) == n
    ohT = np.zeros(nblk_g * 128 * CAP, dtype=f8)
    ohT_idx = ((gb_s * 128 + dstloc) * (CAP // 128) + sb_ // 128) * 128 + sb_ % 128
    ohT[ohT_idx] = 1.0
    ohT = ohT.reshape(nblk_g, 128, CAP // 128, 128)
    # oh[b, e, j, n] = 1 iff dst_local(slot j*128+e of block b) == n
    # pads (dstloc==blk==125) land in discarded output rows 125..127
    oh = np.zeros(nblk_g * 128 * CAP, dtype=f8)
    oh_idx = ((gb_s * 128 + sb_ % 128) * (CAP // 128) + sb_ // 128) * 128 + dstloc
    oh[oh_idx] = 1.0
    oh = oh.reshape(nblk_g, 128, CAP // 128, 128)

    per_core = nblk * CAP
    in_maps = []
    for c_i in range(cores):
        bsl = slice(c_i * nblk, (c_i + 1) * nblk)
        sl = slice(c_i * per_core, (c_i + 1) * per_core)
        m = {
            "fsT": np.ascontiguousarray(fs_pad[sl].T),
            "efT": np.ascontiguousarray(ef_pad[sl].T),
            "ohT": np.ascontiguousarray(ohT[bsl]),
            "oh": np.ascontiguousarray(oh[bsl]),
            "featT": np.ascontiguousarray(
                featp[c_i * npc : (c_i + 1) * npc].T.astype(np.float16)
            ),
            "WoT": np.ascontiguousarray(np.asarray(inputs["Wo"]).T),
            "WskipT": np.ascontiguousarray(np.asarray(inputs["Wskip"]).T),
            "Weg16": np.ascontiguousarray(
                np.concatenate(
                    [np.asarray(inputs["We"]), np.asarray(inputs["Wg"])], axis=1
                ).astype(np.float16)
            ),
            "Wgate": np.ascontiguousarray(np.asarray(inputs["Wgate"])),
        }
        for name in ("Wq", "Wk", "Wv", "Wo", "Wskip", "W1", "W2"):
            m[name + "16"] = np.ascontiguousarray(
                np.asarray(inputs[name]).astype(np.float16)
            )
        for name in ("ln1_g", "ln1_b", "ln2_g", "ln2_b"):
            m[name] = np.ascontiguousarray(np.asarray(inputs[name]))
        in_maps.append(m)
    return in_maps


_cache = {}


def _get_program(cfg):
    key = (cfg["cores"], cfg["n_nodes"], cfg["M"], TREE_GPS, GPS_PW)
    if key not in _cache:
        _cache[key] = build_program(cfg)
    return _cache[key]


def full_base():
    return dict(cores=CORES, n_nodes=N_NODES, npc=NPC, nblk=NBLK, blk=BLK)


def _ensure_ntff_hook():
    import types

    if "antenv.axon_hooks" in sys.modules:
        return
    try:
        sys.path.insert(0, "/root/.axon_site")
        from trn_agent_boot.trn_boot import _ntff_profile_via_ctypes

        hook = _ntff_profile_via_ctypes("/opt/axon/libaxon_pjrt.so")
        mod = types.ModuleType("antenv.axon_hooks")
        mod.get_axon_ntff_profile_hook = lambda: hook
        mod.set_axon_ntff_profile_hook = lambda h: None
        sys.modules["antenv.axon_hooks"] = mod
    except Exception as e:
        print(f"ntff hook setup failed: {e}")


def run(inputs, trace=False, tmpdir=None, trace_cores=None):
    if trace:
        _ensure_ntff_hook()
    cfg, layout = compute_layout(inputs, full_base())
    nc = _get_program(cfg)
    in_maps = shard_inputs(inputs, cfg, layout)
    res = bass_utils.run_bass_kernel_spmd(
        nc,
        in_maps,
        core_ids=list(range(cfg["cores"])),
        trace=trace,
        tmpdir=tmpdir,
        trace_cores=trace_cores,
    )
    out = np.concatenate([res.results[c]["out"] for c in range(cfg["cores"])], axis=0)
    out = out[layout["newid"]]
    return out, res


def kernel(**inputs):
    out, _ = run(inputs)
    return out


# revision 28
# speedup vs baseline: 1.7952x; 1.0002x over previous
"""Trainium2 Bass kernel for GAT-style GNN message passing (edge softmax).

Contract: kernel(**inputs) takes FULL unsharded numpy inputs, distributes
across 8 NeuronCores internally, returns FULL output.

Sharding: edges sorted by dst and partitioned by dst range (6250 nodes per
core) -> every per-destination segment reduction is core-local. Node
features/weights replicated.

v2 design (single fused pass over 50 dst-blocks of 125 nodes):
- host ships fslotT/edge_featT in f16 (halves the two dominant DMA streams,
  removes on-chip casts) and BOTH one-hot orientations (oh + ohT) as f8.
- per-block, per 4-tile group: q/k/eg matmuls into PSUM, qk product read
  directly from PSUM (no PSUM->SBUF drains of q/k/v at all).
- head-dim dot via f16 2x tree-reduce instead of tensor_reduce.
- exp values kept in bf16 (same exponent range as f32) so the aggregation
  matmul runs at 1 cy/row instead of fp32's 4.
- second PE pass recomputes v per group; pu=v*pg multiplied straight from
  PSUM; agg matmul lhsT is the host-shipped f8 one-hot.
- epilogue (gated residual + 2xLN + FFN) as baseline but f16 weights.
"""

import os
import sys

sys.path.insert(0, "/opt/trn_rl_repo")

import numpy as np

import concourse.bass as bass
import concourse.mybir as mybir
import concourse.tile as tile
from concourse import bacc
from concourse import bass_utils
from concourse.masks import make_identity

F32 = mybir.dt.float32
F16 = mybir.dt.float16
BF16 = mybir.dt.bfloat16
F8 = mybir.dt.float8e4
I32 = mybir.dt.int32
AF = mybir.ActivationFunctionType
OP = mybir.AluOpType

D = 128
H = 8
HD = 16
EPS = 1e-5

N_NODES = 50000
N_EDGES = 800000
CORES = 8
NPC = N_NODES // CORES      # nodes per core = 6250
BLK = 125                   # dst nodes per block
NBLK = NPC // BLK           # 50 blocks per core
G = 4                       # edge tiles per PSUM group

# gpsimd cannot access PSUM; it gets the SBUF-only elementwise work
TREE_GPS = int(os.environ.get("KERNEL_TREE_GPS", "0"))
GPS_PW = int(os.environ.get("KERNEL_GPS_PW", "0"))


def _force_act_set():
    """Pin every ACTIVATE to the natural_log_exp_and_others table so the
    kernel pays one ACT_TABLE_LOAD instead of hundreds."""
    from concourse import hw_specs

    if getattr(bacc, "_act_set_forced", False):
        return
    real = hw_specs.get_activation_tables

    def patched(arch):
        t = dict(real(arch))
        keep = "natural_log_exp_and_others"
        return {name: (fns if name == keep else set()) for name, fns in t.items()}

    bacc.get_activation_tables = patched
    bacc._act_set_forced = True


def build_program(cfg):
    _force_act_set()
    cores = cfg["cores"]
    npc = cfg["npc"]
    nblk = cfg["nblk"]
    blk = cfg["blk"]
    M = cfg["M"]
    CAP = M * 128
    EPC = nblk * CAP

    nc = bacc.Bacc(
        "TRN2", target_bir_lowering=False, debug=False, num_devices=cores
    )

    # ---- I/O ----
    fsT_d = nc.dram_tensor("fsT", [D, EPC], F16, kind="ExternalInput").ap()
    efT_d = nc.dram_tensor("efT", [D, EPC], F16, kind="ExternalInput").ap()
    ohT_d = nc.dram_tensor("ohT", [nblk, 128, M, 128], F8, kind="ExternalInput").ap()
    oh_d = nc.dram_tensor("oh", [nblk, 128, M, 128], F8, kind="ExternalInput").ap()
    featT_d = nc.dram_tensor("featT", [D, npc], F16, kind="ExternalInput").ap()
    w_in = {}
    for name in ("Wq16", "Wk16", "Wv16", "Wo16", "Wskip16", "W116", "W216"):
        w_in[name] = nc.dram_tensor(name, [D, D], F16, kind="ExternalInput").ap()
    for name in ("WoT", "WskipT"):
        w_in[name] = nc.dram_tensor(name, [D, D], F32, kind="ExternalInput").ap()
    w_in["Weg16"] = nc.dram_tensor("Weg16", [D, 2 * H], F16, kind="ExternalInput").ap()
    w_in["Wgate"] = nc.dram_tensor("Wgate", [3 * D, 1], F32, kind="ExternalInput").ap()
    for name in ("ln1_g", "ln1_b", "ln2_g", "ln2_b"):
        w_in[name] = nc.dram_tensor(name, [D], F32, kind="ExternalInput").ap()
    out_d = nc.dram_tensor("out", [npc, D], F32, kind="ExternalOutput").ap()
    DBG = int(os.environ.get("KERNEL_DBG", "0"))
    if DBG:
        dbg_agg = nc.dram_tensor("dbg_agg", [nblk, 128, H + D], F32,
                                 kind="ExternalOutput").ap()
        dbg_qk = nc.dram_tensor("dbg_qk", [nblk, 128, M * 128], F32,
                                kind="ExternalOutput").ap()
        dbg_a = nc.dram_tensor("dbg_a", [nblk, 128, M * H], F32,
                               kind="ExternalOutput").ap()
        dbg_pu = nc.dram_tensor("dbg_pu", [nblk, 128, M * (H + D)], F32,
                                kind="ExternalOutput").ap()

    with tile.TileContext(nc) as tc:
        import contextlib

        ctx = contextlib.ExitStack()
        with ctx:
            consts = ctx.enter_context(tc.tile_pool(name="consts", bufs=1))

            # ---------- setup ----------
            ident = consts.tile([128, 128], F32)
            make_identity(nc, ident[:])

            ones_row = consts.tile([1, 128], F32)
            nc.vector.memset(ones_row[:], 1.0)

            const2 = consts.tile([128, 2], F32)
            nc.vector.memset(const2[:, 0:1], 0.0)
            nc.vector.memset(const2[:, 1:2], EPS)
            nc.const_aps.aps[(F32, 0.0)] = const2[:, 0:1]
            nc.const_aps.aps[(F32, EPS)] = const2[:, 1:2]

            lnrow = consts.tile([1, 4 * D], F32)
            for i, name in enumerate(("ln1_g", "ln1_b", "ln2_g", "ln2_b")):
                nc.sync.dma_start(
                    out=lnrow[:, i * D : (i + 1) * D], in_=w_in[name][None, :]
                )
            lnb = consts.tile([128, 4 * D], F32)

            wq16 = consts.tile([D, D], F16)
            nc.sync.dma_start(out=wq16[:], in_=w_in["Wq16"][:])
            wv16 = consts.tile([D, D], F16)
            nc.sync.dma_start(out=wv16[:], in_=w_in["Wv16"][:])
            wk16 = consts.tile([D, D], F16)
            nc.sync.dma_start(out=wk16[:], in_=w_in["Wk16"][:])
            weg16 = consts.tile([D, 2 * H], F16)
            nc.sync.dma_start(out=weg16[:], in_=w_in["Weg16"][:])
            w1_16 = consts.tile([D, D], F16)
            nc.sync.dma_start(out=w1_16[:], in_=w_in["W116"][:])
            w2_16 = consts.tile([D, D], F16)
            nc.sync.dma_start(out=w2_16[:], in_=w_in["W216"][:])

            # gate vector folding: gate_pre = agg@(Wo@A) + feat@(Wskip@B)
            wg3 = consts.tile([128, 3], F32)
            nc.sync.dma_start(
                out=wg3[:], in_=w_in["Wgate"].rearrange("(t p) c -> p (t c)", p=128)
            )
            ab = consts.tile([128, 2], F32)
            nc.vector.tensor_add(out=ab[:, 0:1], in0=wg3[:, 0:1], in1=wg3[:, 2:3])
            nc.vector.tensor_sub(out=ab[:, 1:2], in0=wg3[:, 1:2], in1=wg3[:, 2:3])

            wot_s = consts.tile([D, D], F32)
            nc.sync.dma_start(out=wot_s[:], in_=w_in["WoT"][:])
            wskipt_s = consts.tile([D, D], F32)
            nc.sync.dma_start(out=wskipt_s[:], in_=w_in["WskipT"][:])

            rhs_o = consts.tile([D, D + 1], F16)
            nc.sync.dma_start(out=rhs_o[:, 0:D], in_=w_in["Wo16"][:])
            rhs_s = consts.tile([D, D + 1], F16)
            nc.sync.dma_start(out=rhs_s[:, 1 : D + 1], in_=w_in["Wskip16"][:])

            featT = consts.tile([D, npc], F16)
            nc.sync.dma_start(out=featT[:], in_=featT_d[:])

            k_all = consts.tile([128, nblk, D], F16)

            with tc.tile_pool(name="psum_setup", bufs=1, space="PSUM") as pss:
                ps_ln = pss.tile([128, 4 * D], F32, tag="ln")
                nc.tensor.matmul(
                    out=ps_ln[:], lhsT=ones_row[:], rhs=lnrow[:], start=True, stop=True
                )
                nc.vector.tensor_copy(out=lnb[:], in_=ps_ln[:])

                ps_c = pss.tile([128, 2], F32, tag="c")
                nc.tensor.matmul(
                    out=ps_c[:, 0:1], lhsT=wot_s[:], rhs=ab[:, 0:1],
                    start=True, stop=True,
                )
                nc.tensor.matmul(
                    out=ps_c[:, 1:2], lhsT=wskipt_s[:], rhs=ab[:, 1:2],
                    start=True, stop=True,
                )
                nc.vector.tensor_copy(out=rhs_o[:, D : D + 1], in_=ps_c[:, 0:1])
                nc.vector.tensor_copy(out=rhs_s[:, 0:1], in_=ps_c[:, 1:2])

            # ---------- stage 1: k for own nodes, kept resident in SBUF ----------
            with tc.tile_pool(name="k_ps", bufs=2, space="PSUM") as kps:
                for b in range(nblk):
                    ps_kb = kps.tile([128, D], F32, tag="kb")
                    nc.tensor.matmul(
                        out=ps_kb[:blk],
                        lhsT=featT[:, b * blk : (b + 1) * blk],
                        rhs=wk16[:],
                        start=True, stop=True,
                    )
                    nc.scalar.activation(
                        out=k_all[:blk, b, :], in_=ps_kb[:blk], func=AF.Copy
                    )

            # ---------- main loop ----------
            sbA = ctx.enter_context(tc.tile_pool(name="sbA", bufs=2))
            sbB = ctx.enter_context(tc.tile_pool(name="sbB", bufs=2))
            epi = ctx.enter_context(tc.tile_pool(name="epi", bufs=2))
            # bank budget (8): q/v pool 3 + k 2 + agg 1 + eg 1 + epi 1.
            # the agg accumulation group must own its bank exclusively: a
            # start=True matmul clears has_written bits for its whole 2KB
            # zero region, which would turn pending accumulates into
            # overwrites (single start+stop matmuls are safe to co-locate).
            ps_qp = ctx.enter_context(tc.tile_pool(name="ps_q", bufs=3, space="PSUM"))
            ps_kp = ctx.enter_context(tc.tile_pool(name="ps_k", bufs=2, space="PSUM"))
            ps_aggp = ctx.enter_context(tc.tile_pool(name="ps_agg", bufs=1, space="PSUM"))
            ps_egp = ctx.enter_context(tc.tile_pool(name="ps_eg", bufs=1, space="PSUM"))
            ps_epip = ctx.enter_context(tc.tile_pool(name="ps_epi", bufs=1, space="PSUM"))

            def layer_norm(x_t, g_col, b_col, out_t, nb):
                nm = epi.tile([blk, 1], F32, tag="ln_nm")
                nc.vector.tensor_reduce(
                    out=nm[:nb], in_=x_t[:nb], axis=mybir.AxisListType.X,
                    op=OP.add, negate=True,
                )
                nm2 = epi.tile([blk, 1], F32, tag="ln_nm2")
                nc.scalar.activation(
                    out=nm2[:nb], in_=nm[:nb], func=AF.Copy, scale=1.0 / D
                )
                xc = epi.tile([blk, D], F32, tag="ln_xc")
                nc.scalar.activation(
                    out=xc[:nb], in_=x_t[:nb], func=AF.Identity, bias=nm2[:nb, 0:1]
                )
                sqd = epi.tile([blk, D], F32, tag="ln_sqd")
                v2 = epi.tile([blk, 1], F32, tag="ln_v2")
                nc.scalar.activation(
                    out=sqd[:nb], in_=xc[:nb], func=AF.Square, accum_out=v2[:nb]
                )
                sd = epi.tile([blk, 1], F32, tag="ln_sd")
                nc.scalar.activation(
                    out=sd[:nb], in_=v2[:nb], func=AF.Ln, scale=1.0 / D, bias=EPS
                )
                rstd = epi.tile([blk, 1], F32, tag="ln_rstd")
                nc.scalar.activation(
                    out=rstd[:nb], in_=sd[:nb], func=AF.Exp, scale=-0.5
                )
                nc.vector.scalar_tensor_tensor(
                    out=out_t[:nb], in0=xc[:nb], scalar=rstd[:nb, 0:1],
                    in1=lnb[:nb, g_col * D : (g_col + 1) * D],
                    op0=OP.mult, op1=OP.mult,
                )
                nc.vector.tensor_add(
                    out=out_t[:nb], in0=out_t[:nb],
                    in1=lnb[:nb, b_col * D : (b_col + 1) * D],
                )

            ngroups = (M + G - 1) // G

            for b in range(nblk):
                # --- loads ---
                fs16 = sbA.tile([128, CAP], F16, tag="fs")
                nc.sync.dma_start(out=fs16[:], in_=fsT_d[:, b * CAP : (b + 1) * CAP])
                ef16 = sbA.tile([128, CAP], F16, tag="ef")
                nc.scalar.dma_start(out=ef16[:], in_=efT_d[:, b * CAP : (b + 1) * CAP])
                ohT_t = sbA.tile([128, M, 128], F8, tag="ohT")
                nc.sync.dma_start(out=ohT_t[:], in_=ohT_d[b])
                oh_t = sbA.tile([128, M, 128], F8, tag="oh")
                nc.scalar.dma_start(out=oh_t[:], in_=oh_d[b])

                ps_agg = ps_aggp.tile([128, H + D], F32, tag="agg")
                ps_eg = ps_egp.tile([128, M, 2 * H], F32, tag="eg")

                qk_t = sbB.tile([128, M, 128], F16, tag="qk")
                k16 = sbB.tile([128, M, 128], F16, tag="k16")

                # --- pass A: q, k, eg matmuls + qk product per group ---
                for g0 in range(0, M, G):
                    ng = min(G, M - g0)
                    ps_q = ps_qp.tile([128, G, 128], F32, tag="q")
                    ps_k = ps_kp.tile([128, G, 128], F32, tag="k")
                    for jj in range(ng):
                        j = g0 + jj
                        nc.tensor.matmul(
                            out=ps_q[:, jj, :],
                            lhsT=fs16[:, j * 128 : (j + 1) * 128],
                            rhs=wq16[:],
                            start=True, stop=True,
                        )
                        nc.tensor.matmul(
                            out=ps_k[:, jj, :],
                            lhsT=ohT_t[:blk, j, :],
                            rhs=k_all[:blk, b, :],
                            start=True, stop=True,
                        )
                        nc.tensor.matmul(
                            out=ps_eg[:, j, :],
                            lhsT=ef16[:, j * 128 : (j + 1) * 128],
                            rhs=weg16[:],
                            start=True, stop=True,
                        )
                    # <=1 PSUM input per DVE op: drain k to SBUF f16 first
                    nc.scalar.activation(
                        out=k16[:, g0 : g0 + ng, :], in_=ps_k[:, 0:ng, :],
                        func=AF.Copy,
                    )
                    nc.vector.tensor_tensor(
                        out=qk_t[:, g0 : g0 + ng, :],
                        in0=ps_q[:, 0:ng, :],
                        in1=k16[:, g0 : g0 + ng, :],
                        op=OP.mult,
                    )

                # --- pass B: block-level pointwise ---
                # head-dim dot as a pairwise tree; level 1 (the big one) on
                # the otherwise-idle gpsimd, the rest on vector
                teng = nc.gpsimd if TREE_GPS else nc.vector
                qk3 = qk_t[:].rearrange("p m (h x) -> p (m h) x", x=HD)
                t1 = sbB.tile([128, M * H, 8], F16, tag="t1")
                teng.tensor_tensor(
                    out=t1[:], in0=qk3[:, :, 0:8], in1=qk3[:, :, 8:16], op=OP.add
                )
                t2 = sbB.tile([128, M * H, 4], F16, tag="t2")
                nc.vector.tensor_add(
                    out=t2[:], in0=t1[:, :, 0:4], in1=t1[:, :, 4:8]
                )
                t3 = sbB.tile([128, M * H, 2], F16, tag="t3")
                nc.vector.tensor_add(
                    out=t3[:], in0=t2[:, :, 0:2], in1=t2[:, :, 2:4]
                )
                a_t = sbB.tile([128, M * H], F16, tag="a")
                nc.vector.tensor_add(
                    out=a_t[:],
                    in0=t3[:].rearrange("p f two -> p (f two)")[:, 0::2],
                    in1=t3[:].rearrange("p f two -> p (f two)")[:, 1::2],
                )
                w_t = sbB.tile([128, M * H], F16, tag="w")
                peng = nc.gpsimd if GPS_PW else nc.vector
                peng.tensor_scalar(
                    out=w_t[:], in0=a_t[:], scalar1=5.0, scalar2=-5.0,
                    op0=OP.min, op1=OP.max,
                )
                wv = w_t[:].rearrange("p (m h) -> p m h", h=H)
                nc.vector.tensor_add(out=wv, in0=wv, in1=ps_eg[:, :, 0:H])

                pu_t = sbB.tile([128, M, H + D], BF16, tag="pu")
                nc.scalar.activation(
                    out=pu_t[:, :, 0:H], in_=wv, func=AF.Exp, scale=4.0
                )
                gex = sbB.tile([128, M, H], F32, tag="gex")
                nc.scalar.activation(
                    out=gex[:], in_=ps_eg[:, :, H : 2 * H], func=AF.Exp, scale=-1.0
                )
                peng.tensor_scalar_add(out=gex[:], in0=gex[:], scalar1=1.0)
                ginv = sbB.tile([128, M, H], F32, tag="ginv")
                nc.vector.reciprocal_approx_fast(
                    out=ginv[:].rearrange("p m h -> p (m h)"),
                    in_=gex[:].rearrange("p m h -> p (m h)"),
                )
                pg = sbB.tile([128, M, H], BF16, tag="pg")
                peng.tensor_tensor(
                    out=pg[:], in0=pu_t[:, :, 0:H], in1=ginv[:], op=OP.mult
                )

                # --- pass C: v matmuls, pu product, aggregation ---
                for g0 in range(0, M, G):
                    ng = min(G, M - g0)
                    ps_v = ps_qp.tile([128, G, 128], F32, tag="q")
                    for jj in range(ng):
                        j = g0 + jj
                        nc.tensor.matmul(
                            out=ps_v[:, jj, :],
                            lhsT=fs16[:, j * 128 : (j + 1) * 128],
                            rhs=wv16[:],
                            start=True, stop=True,
                        )
                    nc.vector.tensor_tensor(
                        out=pu_t[:, g0 : g0 + ng, H : H + D].rearrange(
                            "p m (h x) -> p m h x", x=HD
                        ),
                        in0=ps_v[:, 0:ng, :].rearrange("p m (h x) -> p m h x", x=HD),
                        in1=pg[:, g0 : g0 + ng, :, None].to_broadcast(
                            [128, ng, H, HD]
                        ),
                        op=OP.mult,
                    )
                    for jj in range(ng):
                        j = g0 + jj
                        nc.tensor.matmul(
                            out=ps_agg[:],
                            lhsT=oh_t[:, j, :],
                            rhs=pu_t[:, j, :],
                            start=(j == 0),
                            stop=(j == M - 1),
                        )

                if DBG & 1:
                    dqk = epi.tile([128, M * 128], F32, tag="dqk")
                    nc.vector.tensor_copy(out=dqk[:], in_=qk_t[:].rearrange("p m x -> p (m x)"))
                    nc.sync.dma_start(out=dbg_qk[b], in_=dqk[:])
                if DBG & 2:
                    da = epi.tile([128, M * H], F32, tag="da")
                    nc.vector.tensor_copy(out=da[:], in_=a_t[:])
                    nc.sync.dma_start(out=dbg_a[b], in_=da[:])
                if DBG & 4:
                    dpu = epi.tile([128, M * (H + D)], F32, tag="dpu")
                    nc.vector.tensor_copy(out=dpu[:], in_=pu_t[:].rearrange("p m c -> p (m c)"))
                    nc.sync.dma_start(out=dbg_pu[b], in_=dpu[:])
                if DBG & 8:
                    dagg = epi.tile([128, H + D], F32, tag="dagg")
                    nc.vector.tensor_copy(out=dagg[:], in_=ps_agg[:])
                    nc.sync.dma_start(out=dbg_agg[b], in_=dagg[:])

                # --- epilogue for this block ---
                nb = blk
                # ps_epi: [0:2D+2) rsf, [2D+2:2D+2+blk) transpose scratch
                ps_epi = ps_epip.tile([128, 2 * D + 2 + blk], F32, tag="epi")
                TR0 = 2 * D + 2

                dsafe = epi.tile([blk, H], F32, tag="ds")
                nc.vector.tensor_scalar_max(
                    out=dsafe[:nb], in0=ps_agg[:nb, 0:H], scalar1=1e-30
                )
                dinv = epi.tile([blk, H], F32, tag="dinv")
                nc.vector.reciprocal(out=dinv[:nb], in_=dsafe[:nb])
                agg_s = epi.tile([blk, D], F32, tag="aggs")
                nc.vector.tensor_mul(
                    out=agg_s[:nb].rearrange("p (h x) -> p h x", x=HD),
                    in0=ps_agg[:nb, H : H + D].rearrange("p (h x) -> p h x", x=HD),
                    in1=dinv[:nb, :, None].to_broadcast([nb, H, HD]),
                )

                nc.tensor.transpose(
                    out=ps_epi[:, TR0 : TR0 + blk], in_=agg_s[:nb],
                    identity=ident[:nb, :nb],
                )
                aggT = epi.tile([D, blk], F16, tag="aggT")
                nc.scalar.activation(
                    out=aggT[:], in_=ps_epi[:, TR0 : TR0 + blk], func=AF.Copy
                )

                nc.tensor.matmul(
                    out=ps_epi[:nb, 0 : D + 1], lhsT=aggT[:, :nb], rhs=rhs_o[:],
                    start=True, stop=True,
                )
                nc.tensor.matmul(
                    out=ps_epi[:nb, D + 1 : 2 * D + 2],
                    lhsT=featT[:, b * blk : b * blk + nb],
                    rhs=rhs_s[:],
                    start=True, stop=True,
                )
                sk_s = epi.tile([blk, D + 1], F32, tag="sk")
                nc.scalar.activation(
                    out=sk_s[:nb], in_=ps_epi[:nb, D + 1 : 2 * D + 2], func=AF.Copy
                )
                gp = epi.tile([blk, 1], F32, tag="gp")
                nc.vector.tensor_add(
                    out=gp[:nb], in0=ps_epi[:nb, D : D + 1], in1=sk_s[:nb, 0:1]
                )
                g_s = epi.tile([blk, 1], F32, tag="g")
                nc.scalar.activation(
                    out=g_s[:nb], in_=gp[:nb], func=AF.Exp, scale=-1.0
                )
                nc.vector.tensor_scalar_add(out=g_s[:nb], in0=g_s[:nb], scalar1=1.0)
                nc.vector.reciprocal(out=g_s[:nb], in_=g_s[:nb])
                diff = epi.tile([blk, D], F32, tag="diff")
                nc.vector.tensor_sub(
                    out=diff[:nb], in0=ps_epi[:nb, 0:D], in1=sk_s[:nb, 1 : D + 1]
                )
                mix = epi.tile([blk, D], F32, tag="mix")
                nc.vector.scalar_tensor_tensor(
                    out=mix[:nb], in0=diff[:nb], scalar=g_s[:nb, 0:1],
                    in1=sk_s[:nb, 1 : D + 1],
                    op0=OP.mult, op1=OP.add,
                )

                h_t = epi.tile([blk, D], F32, tag="h")
                layer_norm(mix, 0, 1, h_t, nb)
                l2 = epi.tile([blk, D], F32, tag="l2")
                layer_norm(h_t, 2, 3, l2, nb)

                nc.tensor.transpose(
                    out=ps_epi[:, TR0 : TR0 + blk], in_=l2[:nb],
                    identity=ident[:nb, :nb],
                )
                l2T = epi.tile([D, blk], F16, tag="l2T")
                nc.scalar.activation(
                    out=l2T[:], in_=ps_epi[:, TR0 : TR0 + blk], func=AF.Copy
                )
                nc.tensor.matmul(
                    out=ps_epi[:nb, D + 2 : 2 * D + 2], lhsT=l2T[:, :nb],
                    rhs=w1_16[:],
                    start=True, stop=True,
                )
                r_t = epi.tile([blk, D], F32, tag="r")
                nc.scalar.activation(
                    out=r_t[:nb], in_=ps_epi[:nb, D + 2 : 2 * D + 2], func=AF.Relu
                )
                nc.tensor.transpose(
                    out=ps_epi[:, TR0 : TR0 + blk], in_=r_t[:nb],
                    identity=ident[:nb, :nb],
                )
                rT = epi.tile([D, blk], F16, tag="rT")
                nc.scalar.activation(
                    out=rT[:], in_=ps_epi[:, TR0 : TR0 + blk], func=AF.Copy
                )
                nc.tensor.matmul(
                    out=ps_epi[:nb, 0:D], lhsT=rT[:, :nb], rhs=w2_16[:],
                    start=True, stop=True,
                )
                outb = epi.tile([blk, D], F32, tag="outb")
                nc.vector.tensor_add(
                    out=outb[:nb], in0=h_t[:nb], in1=ps_epi[:nb, 0:D]
                )
                nc.sync.dma_start(
                    out=out_d[b * blk : b * blk + nb, :], in_=outb[:nb]
                )

    nc.compile()
    return nc


def _balance_blocks(deg, nblk, blk):
    """LPT-pack nodes into nblk blocks of exactly blk nodes, equalizing the
    per-block edge load. Returns newid[orig_local] -> new local id."""
    import heapq

    npc = len(deg)
    order = np.argsort(-deg, kind="stable")
    cnt = np.zeros(nblk, dtype=np.int64)
    heap = [(0, b) for b in range(nblk)]
    heapq.heapify(heap)
    newid = np.empty(npc, dtype=np.int64)
    for n in order:
        while True:
            load, b = heapq.heappop(heap)
            if cnt[b] < blk:
                break
        newid[n] = b * blk + cnt[b]
        cnt[b] += 1
        if cnt[b] < blk:
            heapq.heappush(heap, (load + deg[n], b))
    return newid


def compute_layout(inputs, base):
    """Permute nodes within each core so per-block edge loads are balanced
    (lower static block capacity M), then lay edges out by dst block."""
    cores, npc, nblk, blk = base["cores"], base["npc"], base["nblk"], base["blk"]
    nblk_g = cores * nblk

    src = np.asarray(inputs["src"]).astype(np.int64)
    dst = np.asarray(inputs["dst"]).astype(np.int64)

    # per-core node permutation (new local id = block*blk + slot)
    newid = np.empty(cores * npc, dtype=np.int64)
    for c in range(cores):
        deg = np.bincount(dst[(dst >= c * npc) & (dst < (c + 1) * npc)] - c * npc,
                          minlength=npc)
        newid[c * npc : (c + 1) * npc] = c * npc + _balance_blocks(deg, nblk, blk)

    dstp = newid[dst]
    gb_all = dstp // blk
    order = np.lexsort((src, gb_all))
    ds = dstp[order]
    ss = src[order]
    gb = gb_all[order]

    counts = np.bincount(gb, minlength=nblk_g)
    M = max(2, int(np.ceil(counts.max() / 128)))

    starts = np.zeros(nblk_g + 1, dtype=np.int64)
    np.cumsum(counts, out=starts[1:])
    pos = np.arange(len(ds)) - starts[gb]
    slot = gb * (M * 128) + pos

    layout = dict(order=order, ds=ds, ss=ss, gb=gb, slot=slot, newid=newid)
    cfg = dict(base, M=M)
    return cfg, layout


def shard_inputs(inputs, cfg, layout):
    """Host-side layout only (sort/pad/transpose/index/cast, no arithmetic)."""
    cores = cfg["cores"]
    npc = cfg["npc"]
    nblk = cfg["nblk"]
    blk = cfg["blk"]
    M = cfg["M"]
    CAP = M * 128
    nblk_g = cores * nblk

    ds, ss, slot = layout["ds"], layout["ss"], layout["slot"]
    gb = layout["gb"]
    edge_feat = np.asarray(inputs["edge_feat"])
    feat = np.asarray(inputs["feat"])
    featp = np.empty_like(feat)
    featp[layout["newid"]] = feat

    total = nblk_g * CAP
    dstloc = np.full(total, blk, dtype=np.int64)
    dstloc[slot] = ds - gb * blk

    ef_pad = np.zeros((total, D), dtype=np.float16)
    ef_pad[slot] = edge_feat[layout["order"]]
    fs_pad = np.zeros((total, D), dtype=np.float16)
    fs_pad[slot] = feat[ss]

    f8 = mybir.dt.np(F8)
    sb_ = np.arange(total) % CAP
    gb_s = np.arange(total) // CAP
    # ohT[b, n, j, e] = 1 iff dst_local(slot j*128+e of block b) == n
    ohT = np.zeros(nblk_g * 128 * CAP, dtype=f8)
    ohT_idx = ((gb_s * 128 + dstloc) * (CAP // 128) + sb_ // 128) * 128 + sb_ % 128
    ohT[ohT_idx] = 1.0
    ohT = ohT.reshape(nblk_g, 128, CAP // 128, 128)
    # oh[b, e, j, n] = 1 iff dst_local(slot j*128+e of block b) == n
    # pads (dstloc==blk==125) land in discarded output rows 125..127
    oh = np.zeros(nblk_g * 128 * CAP, dtype=f8)
    oh_idx = ((gb_s * 128 + sb_ % 128) * (CAP // 128) + sb_ // 128) * 128 + dstloc
    oh[oh_idx] = 1.0
    oh = oh.reshape(nblk_g, 128, CAP // 128, 128)

    per_core = nblk * CAP
    in_maps = []
    for c_i in range(cores):
        bsl = slice(c_i * nblk, (c_i + 1) * nblk)
        sl = slice(c_i * per_core, (c_i + 1) * per_core)
        m = {
            "fsT": np.ascontiguousarray(fs_pad[sl].T),
            "efT": np.ascontiguousarray(ef_pad[sl].T),
            "ohT": np.ascontiguousarray(ohT[bsl]),
            "oh": np.ascontiguousarray(oh[bsl]),
            "featT": np.ascontiguousarray(
                featp[c_i * npc : (c_i + 1) * npc].T.astype(np.float16)
            ),
            "WoT": np.ascontiguousarray(np.asarray(inputs["Wo"]).T),
            "WskipT": np.ascontiguousarray(np.asarray(inputs["Wskip"]).T),
            "Weg16": np.ascontiguousarray(
                np.concatenate(
                    [np.asarray(inputs["We"]), np.asarray(inputs["Wg"])], axis=1
                ).astype(np.float16)
            ),
            "Wgate": np.ascontiguousarray(np.asarray(inputs["Wgate"])),
        }
        for name in ("Wq", "Wk", "Wv", "Wo", "Wskip", "W1", "W2"):
            m[name + "16"] = np.ascontiguousarray(
                np.asarray(inputs[name]).astype(np.float16)
            )
        for name in ("ln1_g", "ln1_b", "ln2_g", "ln2_b"):
            m[name] = np.ascontiguousarray(np.asarray(inputs[name]))
        in_maps.append(m)
    return in_maps


_cache = {}


def _get_program(cfg):
    key = (cfg["cores"], cfg["n_nodes"], cfg["M"], TREE_GPS, GPS_PW, os.environ.get("KERNEL_DBG","0"))
    if key not in _cache:
        _cache[key] = build_program(cfg)
    return _cache[key]


def full_base():
    return dict(cores=CORES, n_nodes=N_NODES, npc=NPC, nblk=NBLK, blk=BLK)


def _ensure_ntff_hook():
    import types

    if "antenv.axon_hooks" in sys.modules:
        return
    try:
        sys.path.insert(0, "/root/.axon_site")
        from trn_agent_boot.trn_boot import _ntff_profile_via_ctypes

        hook = _ntff_profile_via_ctypes("/opt/axon/libaxon_pjrt.so")
        mod = types.ModuleType("antenv.axon_hooks")
        mod.get_axon_ntff_profile_hook = lambda: hook
        mod.set_axon_ntff_profile_hook = lambda h: None
        sys.modules["antenv.axon_hooks"] = mod
    except Exception as e:
        print(f"ntff hook setup failed: {e}")


def run(inputs, trace=False, tmpdir=None, trace_cores=None):
    if trace:
        _ensure_ntff_hook()
    cfg, layout = compute_layout(inputs, full_base())
    nc = _get_program(cfg)
    in_maps = shard_inputs(inputs, cfg, layout)
    res = bass_utils.run_bass_kernel_spmd(
        nc,
        in_maps,
        core_ids=list(range(cfg["cores"])),
        trace=trace,
        tmpdir=tmpdir,
        trace_cores=trace_cores,
    )
    out = np.concatenate([res.results[c]["out"] for c in range(cfg["cores"])], axis=0)
    out = out[layout["newid"]]
    return out, res


def kernel(**inputs):
    out, _ = run(inputs)
    return out


# revision 29
# speedup vs baseline: 1.9445x; 1.0832x over previous
"""Trainium2 Bass kernel for GAT-style GNN message passing (edge softmax).

Contract: kernel(**inputs) takes FULL unsharded numpy inputs, distributes
across 8 NeuronCores internally, returns FULL output.

Sharding: edges sorted by dst and partitioned by dst range (6250 nodes per
core) -> every per-destination segment reduction is core-local. Node
features/weights replicated.

v2 design (single fused pass over 50 dst-blocks of 125 nodes):
- host ships fslotT/edge_featT in f16 (halves the two dominant DMA streams,
  removes on-chip casts) and BOTH one-hot orientations (oh + ohT) as f8.
- per-block, per 4-tile group: q/k/eg matmuls into PSUM, qk product read
  directly from PSUM (no PSUM->SBUF drains of q/k/v at all).
- head-dim dot via f16 2x tree-reduce instead of tensor_reduce.
- exp values kept in bf16 (same exponent range as f32) so the aggregation
  matmul runs at 1 cy/row instead of fp32's 4.
- second PE pass recomputes v per group; pu=v*pg multiplied straight from
  PSUM; agg matmul lhsT is the host-shipped f8 one-hot.
- epilogue (gated residual + 2xLN + FFN) as baseline but f16 weights.
"""

import os
import sys

sys.path.insert(0, "/opt/trn_rl_repo")

import numpy as np

import concourse.bass as bass
import concourse.mybir as mybir
import concourse.tile as tile
from concourse import bacc
from concourse import bass_utils
from concourse.masks import make_identity

F32 = mybir.dt.float32
F16 = mybir.dt.float16
BF16 = mybir.dt.bfloat16
F8 = mybir.dt.float8e4
I32 = mybir.dt.int32
AF = mybir.ActivationFunctionType
OP = mybir.AluOpType

D = 128
H = 8
HD = 16
EPS = 1e-5

N_NODES = 50000
N_EDGES = 800000
CORES = 8
NPC = N_NODES // CORES      # nodes per core = 6250
BLK = 125                   # dst nodes per block
NBLK = NPC // BLK           # 50 blocks per core
G = 4                       # edge tiles per PSUM group

# gpsimd cannot access PSUM; it gets the SBUF-only elementwise work
TREE_GPS = int(os.environ.get("KERNEL_TREE_GPS", "0"))
GPS_PW = int(os.environ.get("KERNEL_GPS_PW", "0"))


def _force_act_set():
    """Pin every ACTIVATE to the natural_log_exp_and_others table so the
    kernel pays one ACT_TABLE_LOAD instead of hundreds."""
    from concourse import hw_specs

    if getattr(bacc, "_act_set_forced", False):
        return
    real = hw_specs.get_activation_tables

    def patched(arch):
        t = dict(real(arch))
        keep = "natural_log_exp_and_others"
        return {name: (fns if name == keep else set()) for name, fns in t.items()}

    bacc.get_activation_tables = patched
    bacc._act_set_forced = True


def build_program(cfg):
    _force_act_set()
    cores = cfg["cores"]
    npc = cfg["npc"]
    nblk = cfg["nblk"]
    blk = cfg["blk"]
    M = cfg["M"]
    CAP = M * 128
    EPC = nblk * CAP

    nc = bacc.Bacc(
        "TRN2", target_bir_lowering=False, debug=False, num_devices=cores
    )

    # ---- I/O ----
    fsT_d = nc.dram_tensor("fsT", [D, EPC], F16, kind="ExternalInput").ap()
    efT_d = nc.dram_tensor("efT", [D, EPC], F16, kind="ExternalInput").ap()
    ohT_d = nc.dram_tensor("ohT", [nblk, 128, M, 128], F8, kind="ExternalInput").ap()
    oh_d = nc.dram_tensor("oh", [nblk, 128, M, 128], F8, kind="ExternalInput").ap()
    featT_d = nc.dram_tensor("featT", [D, npc], F16, kind="ExternalInput").ap()
    w_in = {}
    for name in ("Wq16", "Wk16", "Wv16", "Wo16", "Wskip16", "W116", "W216"):
        w_in[name] = nc.dram_tensor(name, [D, D], F16, kind="ExternalInput").ap()
    for name in ("WoT", "WskipT"):
        w_in[name] = nc.dram_tensor(name, [D, D], F32, kind="ExternalInput").ap()
    w_in["Weg16"] = nc.dram_tensor("Weg16", [D, 2 * H], F16, kind="ExternalInput").ap()
    w_in["Wgate"] = nc.dram_tensor("Wgate", [3 * D, 1], F32, kind="ExternalInput").ap()
    for name in ("ln1_g", "ln1_b", "ln2_g", "ln2_b"):
        w_in[name] = nc.dram_tensor(name, [D], F32, kind="ExternalInput").ap()
    out_d = nc.dram_tensor("out", [npc, D], F32, kind="ExternalOutput").ap()
    DBG = int(os.environ.get("KERNEL_DBG", "0"))
    if DBG:
        dbg_agg = nc.dram_tensor("dbg_agg", [nblk, 128, H + D], F32,
                                 kind="ExternalOutput").ap()
        dbg_qk = nc.dram_tensor("dbg_qk", [nblk, 128, M * 128], F32,
                                kind="ExternalOutput").ap()
        dbg_a = nc.dram_tensor("dbg_a", [nblk, 128, M * H], F32,
                               kind="ExternalOutput").ap()
        dbg_pu = nc.dram_tensor("dbg_pu", [nblk, 128, M * (H + D)], F32,
                                kind="ExternalOutput").ap()

    with tile.TileContext(nc) as tc:
        import contextlib

        ctx = contextlib.ExitStack()
        with ctx:
            consts = ctx.enter_context(tc.tile_pool(name="consts", bufs=1))

            # ---------- setup ----------
            ident = consts.tile([128, 128], F32)
            make_identity(nc, ident[:])

            ones_row = consts.tile([1, 128], F32)
            nc.vector.memset(ones_row[:], 1.0)

            const2 = consts.tile([128, 2], F32)
            nc.vector.memset(const2[:, 0:1], 0.0)
            nc.vector.memset(const2[:, 1:2], EPS)
            nc.const_aps.aps[(F32, 0.0)] = const2[:, 0:1]
            nc.const_aps.aps[(F32, EPS)] = const2[:, 1:2]

            lnrow = consts.tile([1, 4 * D], F32)
            for i, name in enumerate(("ln1_g", "ln1_b", "ln2_g", "ln2_b")):
                nc.sync.dma_start(
                    out=lnrow[:, i * D : (i + 1) * D], in_=w_in[name][None, :]
                )
            lnb = consts.tile([128, 4 * D], F32)

            wq16 = consts.tile([D, D], F16)
            nc.sync.dma_start(out=wq16[:], in_=w_in["Wq16"][:])
            wv16 = consts.tile([D, D], F16)
            nc.sync.dma_start(out=wv16[:], in_=w_in["Wv16"][:])
            wk16 = consts.tile([D, D], F16)
            nc.sync.dma_start(out=wk16[:], in_=w_in["Wk16"][:])
            weg16 = consts.tile([D, 2 * H], F16)
            nc.sync.dma_start(out=weg16[:], in_=w_in["Weg16"][:])
            w1_16 = consts.tile([D, D], F16)
            nc.sync.dma_start(out=w1_16[:], in_=w_in["W116"][:])
            w2_16 = consts.tile([D, D], F16)
            nc.sync.dma_start(out=w2_16[:], in_=w_in["W216"][:])

            # gate vector folding: gate_pre = agg@(Wo@A) + feat@(Wskip@B)
            wg3 = consts.tile([128, 3], F32)
            nc.sync.dma_start(
                out=wg3[:], in_=w_in["Wgate"].rearrange("(t p) c -> p (t c)", p=128)
            )
            ab = consts.tile([128, 2], F32)
            nc.vector.tensor_add(out=ab[:, 0:1], in0=wg3[:, 0:1], in1=wg3[:, 2:3])
            nc.vector.tensor_sub(out=ab[:, 1:2], in0=wg3[:, 1:2], in1=wg3[:, 2:3])

            wot_s = consts.tile([D, D], F32)
            nc.sync.dma_start(out=wot_s[:], in_=w_in["WoT"][:])
            wskipt_s = consts.tile([D, D], F32)
            nc.sync.dma_start(out=wskipt_s[:], in_=w_in["WskipT"][:])

            rhs_o = consts.tile([D, D + 1], F16)
            nc.sync.dma_start(out=rhs_o[:, 0:D], in_=w_in["Wo16"][:])
            rhs_s = consts.tile([D, D + 1], F16)
            nc.sync.dma_start(out=rhs_s[:, 1 : D + 1], in_=w_in["Wskip16"][:])

            featT = consts.tile([D, npc], F16)
            nc.sync.dma_start(out=featT[:], in_=featT_d[:])

            k_all = consts.tile([128, nblk, D], F16)

            with tc.tile_pool(name="psum_setup", bufs=1, space="PSUM") as pss:
                ps_ln = pss.tile([128, 4 * D], F32, tag="ln")
                nc.tensor.matmul(
                    out=ps_ln[:], lhsT=ones_row[:], rhs=lnrow[:], start=True, stop=True
                )
                nc.vector.tensor_copy(out=lnb[:], in_=ps_ln[:])

                ps_c = pss.tile([128, 2], F32, tag="c")
                nc.tensor.matmul(
                    out=ps_c[:, 0:1], lhsT=wot_s[:], rhs=ab[:, 0:1],
                    start=True, stop=True,
                )
                nc.tensor.matmul(
                    out=ps_c[:, 1:2], lhsT=wskipt_s[:], rhs=ab[:, 1:2],
                    start=True, stop=True,
                )
                nc.vector.tensor_copy(out=rhs_o[:, D : D + 1], in_=ps_c[:, 0:1])
                nc.vector.tensor_copy(out=rhs_s[:, 0:1], in_=ps_c[:, 1:2])

            # ---------- stage 1: k for own nodes, kept resident in SBUF ----------
            with tc.tile_pool(name="k_ps", bufs=2, space="PSUM") as kps:
                for b in range(nblk):
                    ps_kb = kps.tile([128, D], F32, tag="kb")
                    nc.tensor.matmul(
                        out=ps_kb[:blk],
                        lhsT=featT[:, b * blk : (b + 1) * blk],
                        rhs=wk16[:],
                        start=True, stop=True,
                    )
                    nc.scalar.activation(
                        out=k_all[:blk, b, :], in_=ps_kb[:blk], func=AF.Copy
                    )

            # ---------- main loop ----------
            sbA = ctx.enter_context(tc.tile_pool(name="sbA", bufs=2))
            sbB = ctx.enter_context(tc.tile_pool(name="sbB", bufs=2))
            epi = ctx.enter_context(tc.tile_pool(name="epi", bufs=2))
            # bank budget (8): q/v pool 3 + k 2 + agg 1 + eg 1 + epi 1.
            # the agg accumulation group must own its bank exclusively: a
            # start=True matmul clears has_written bits for its whole 2KB
            # zero region, which would turn pending accumulates into
            # overwrites (single start+stop matmuls are safe to co-locate).
            ps_qp = ctx.enter_context(tc.tile_pool(name="ps_q", bufs=3, space="PSUM"))
            ps_kp = ctx.enter_context(tc.tile_pool(name="ps_k", bufs=2, space="PSUM"))
            ps_aggp = ctx.enter_context(tc.tile_pool(name="ps_agg", bufs=1, space="PSUM"))
            ps_egp = ctx.enter_context(tc.tile_pool(name="ps_eg", bufs=1, space="PSUM"))
            ps_epip = ctx.enter_context(tc.tile_pool(name="ps_epi", bufs=1, space="PSUM"))

            def layer_norm(x_t, g_col, b_col, out_t, nb):
                nm = epi.tile([blk, 1], F32, tag="ln_nm")
                nc.vector.tensor_reduce(
                    out=nm[:nb], in_=x_t[:nb], axis=mybir.AxisListType.X,
                    op=OP.add, negate=True,
                )
                nm2 = epi.tile([blk, 1], F32, tag="ln_nm2")
                nc.scalar.activation(
                    out=nm2[:nb], in_=nm[:nb], func=AF.Copy, scale=1.0 / D
                )
                xc = epi.tile([blk, D], F32, tag="ln_xc")
                nc.scalar.activation(
                    out=xc[:nb], in_=x_t[:nb], func=AF.Identity, bias=nm2[:nb, 0:1]
                )
                sqd = epi.tile([blk, D], F32, tag="ln_sqd")
                v2 = epi.tile([blk, 1], F32, tag="ln_v2")
                nc.scalar.activation(
                    out=sqd[:nb], in_=xc[:nb], func=AF.Square, accum_out=v2[:nb]
                )
                sd = epi.tile([blk, 1], F32, tag="ln_sd")
                nc.scalar.activation(
                    out=sd[:nb], in_=v2[:nb], func=AF.Ln, scale=1.0 / D, bias=EPS
                )
                rstd = epi.tile([blk, 1], F32, tag="ln_rstd")
                nc.scalar.activation(
                    out=rstd[:nb], in_=sd[:nb], func=AF.Exp, scale=-0.5
                )
                nc.vector.scalar_tensor_tensor(
                    out=out_t[:nb], in0=xc[:nb], scalar=rstd[:nb, 0:1],
                    in1=lnb[:nb, g_col * D : (g_col + 1) * D],
                    op0=OP.mult, op1=OP.mult,
                )
                nc.vector.tensor_add(
                    out=out_t[:nb], in0=out_t[:nb],
                    in1=lnb[:nb, b_col * D : (b_col + 1) * D],
                )

            ngroups = (M + G - 1) // G

            for b in range(nblk):
                # --- loads ---
                fs16 = sbA.tile([128, CAP], F16, tag="fs")
                nc.sync.dma_start(out=fs16[:], in_=fsT_d[:, b * CAP : (b + 1) * CAP])
                ef16 = sbA.tile([128, CAP], F16, tag="ef")
                nc.scalar.dma_start(out=ef16[:], in_=efT_d[:, b * CAP : (b + 1) * CAP])
                ohT_t = sbA.tile([128, M, 128], F8, tag="ohT")
                nc.sync.dma_start(out=ohT_t[:], in_=ohT_d[b])
                oh_t = sbA.tile([128, M, 128], F8, tag="oh")
                nc.scalar.dma_start(out=oh_t[:], in_=oh_d[b])

                ps_agg = ps_aggp.tile([128, H + D], F32, tag="agg")
                ps_eg = ps_egp.tile([128, M, 2 * H], F32, tag="eg")

                qk_t = sbB.tile([128, M, 128], F16, tag="qk")
                k16 = sbB.tile([128, M, 128], F16, tag="k16")

                # --- pass A: q, k, eg matmuls + qk product per group ---
                for g0 in range(0, M, G):
                    ng = min(G, M - g0)
                    ps_q = ps_qp.tile([128, G, 128], F32, tag="q")
                    ps_k = ps_kp.tile([128, G, 128], F32, tag="k")
                    for jj in range(ng):
                        j = g0 + jj
                        nc.tensor.matmul(
                            out=ps_q[:, jj, :],
                            lhsT=fs16[:, j * 128 : (j + 1) * 128],
                            rhs=wq16[:],
                            start=True, stop=True,
                        )
                        nc.tensor.matmul(
                            out=ps_k[:, jj, :],
                            lhsT=ohT_t[:blk, j, :],
                            rhs=k_all[:blk, b, :],
                            start=True, stop=True,
                        )
                        nc.tensor.matmul(
                            out=ps_eg[:, j, :],
                            lhsT=ef16[:, j * 128 : (j + 1) * 128],
                            rhs=weg16[:],
                            start=True, stop=True,
                        )
                    # <=1 PSUM input per DVE op: drain k to SBUF f16 first
                    nc.scalar.activation(
                        out=k16[:, g0 : g0 + ng, :], in_=ps_k[:, 0:ng, :],
                        func=AF.Copy,
                    )
                    nc.vector.tensor_tensor(
                        out=qk_t[:, g0 : g0 + ng, :],
                        in0=ps_q[:, 0:ng, :],
                        in1=k16[:, g0 : g0 + ng, :],
                        op=OP.mult,
                    )

                # --- pass B: block-level pointwise ---
                # head-dim dot as a pairwise tree; level 1 (the big one) on
                # the otherwise-idle gpsimd, the rest on vector
                teng = nc.gpsimd if TREE_GPS else nc.vector
                qk3 = qk_t[:].rearrange("p m (h x) -> p (m h) x", x=HD)
                t1 = sbB.tile([128, M * H, 8], F16, tag="t1")
                teng.tensor_tensor(
                    out=t1[:], in0=qk3[:, :, 0:8], in1=qk3[:, :, 8:16], op=OP.add
                )
                t2 = sbB.tile([128, M * H, 4], F16, tag="t2")
                nc.vector.tensor_add(
                    out=t2[:], in0=t1[:, :, 0:4], in1=t1[:, :, 4:8]
                )
                t3 = sbB.tile([128, M * H, 2], F16, tag="t3")
                nc.vector.tensor_add(
                    out=t3[:], in0=t2[:, :, 0:2], in1=t2[:, :, 2:4]
                )
                a_t = sbB.tile([128, M * H], F16, tag="a")
                nc.vector.tensor_add(
                    out=a_t[:],
                    in0=t3[:].rearrange("p f two -> p (f two)")[:, 0::2],
                    in1=t3[:].rearrange("p f two -> p (f two)")[:, 1::2],
                )
                w_t = sbB.tile([128, M * H], F16, tag="w")
                peng = nc.gpsimd if GPS_PW else nc.vector
                peng.tensor_scalar(
                    out=w_t[:], in0=a_t[:], scalar1=5.0, scalar2=-5.0,
                    op0=OP.min, op1=OP.max,
                )
                wv = w_t[:].rearrange("p (m h) -> p m h", h=H)
                nc.vector.tensor_add(out=wv, in0=wv, in1=ps_eg[:, :, 0:H])

                pu_t = sbB.tile([128, M, H + D], BF16, tag="pu")
                nc.scalar.activation(
                    out=pu_t[:, :, 0:H], in_=wv, func=AF.Exp, scale=4.0
                )
                gex = sbB.tile([128, M, H], F32, tag="gex")
                nc.scalar.activation(
                    out=gex[:], in_=ps_eg[:, :, H : 2 * H], func=AF.Exp, scale=-1.0
                )
                peng.tensor_scalar_add(out=gex[:], in0=gex[:], scalar1=1.0)
                ginv = sbB.tile([128, M, H], F32, tag="ginv")
                nc.vector.reciprocal_approx_fast(
                    out=ginv[:].rearrange("p m h -> p (m h)"),
                    in_=gex[:].rearrange("p m h -> p (m h)"),
                )
                pg = sbB.tile([128, M, H], BF16, tag="pg")
                peng.tensor_tensor(
                    out=pg[:], in0=pu_t[:, :, 0:H], in1=ginv[:], op=OP.mult
                )

                # --- pass C: v matmuls, pu product, aggregation ---
                for g0 in range(0, M, G):
                    ng = min(G, M - g0)
                    ps_v = ps_qp.tile([128, G, 128], F32, tag="q")
                    for jj in range(ng):
                        j = g0 + jj
                        nc.tensor.matmul(
                            out=ps_v[:, jj, :],
                            lhsT=fs16[:, j * 128 : (j + 1) * 128],
                            rhs=wv16[:],
                            start=True, stop=True,
                        )
                    nc.vector.tensor_tensor(
                        out=pu_t[:, g0 : g0 + ng, H : H + D].rearrange(
                            "p m (h x) -> p m h x", x=HD
                        ),
                        in0=ps_v[:, 0:ng, :].rearrange("p m (h x) -> p m h x", x=HD),
                        in1=pg[:, g0 : g0 + ng, :, None].to_broadcast(
                            [128, ng, H, HD]
                        ),
                        op=OP.mult,
                    )
                    for jj in range(ng):
                        j = g0 + jj
                        nc.tensor.matmul(
                            out=ps_agg[:],
                            lhsT=oh_t[:, j, :],
                            rhs=pu_t[:, j, :],
                            start=(j == 0),
                            stop=(j == M - 1),
                        )

                if DBG & 1:
                    dqk = epi.tile([128, M * 128], F32, tag="dqk")
                    nc.vector.tensor_copy(out=dqk[:], in_=qk_t[:].rearrange("p m x -> p (m x)"))
                    nc.sync.dma_start(out=dbg_qk[b], in_=dqk[:])
                if DBG & 2:
                    da = epi.tile([128, M * H], F32, tag="da")
                    nc.vector.tensor_copy(out=da[:], in_=a_t[:])
                    nc.sync.dma_start(out=dbg_a[b], in_=da[:])
                if DBG & 4:
                    dpu = epi.tile([128, M * (H + D)], F32, tag="dpu")
                    nc.vector.tensor_copy(out=dpu[:], in_=pu_t[:].rearrange("p m c -> p (m c)"))
                    nc.sync.dma_start(out=dbg_pu[b], in_=dpu[:])
                if DBG & 8:
                    dagg = epi.tile([128, H + D], F32, tag="dagg")
                    nc.vector.tensor_copy(out=dagg[:], in_=ps_agg[:])
                    nc.sync.dma_start(out=dbg_agg[b], in_=dagg[:])

                # --- epilogue for this block ---
                nb = blk
                # ps_epi: [0:2D+2) rsf, [2D+2:2D+2+blk) transpose scratch
                ps_epi = ps_epip.tile([128, 2 * D + 2 + blk], F32, tag="epi")
                TR0 = 2 * D + 2

                dsafe = epi.tile([blk, H], F32, tag="ds")
                nc.vector.tensor_scalar_max(
                    out=dsafe[:nb], in0=ps_agg[:nb, 0:H], scalar1=1e-30
                )
                dinv = epi.tile([blk, H], F32, tag="dinv")
                nc.vector.reciprocal(out=dinv[:nb], in_=dsafe[:nb])
                agg_s = epi.tile([blk, D], F32, tag="aggs")
                nc.vector.tensor_mul(
                    out=agg_s[:nb].rearrange("p (h x) -> p h x", x=HD),
                    in0=ps_agg[:nb, H : H + D].rearrange("p (h x) -> p h x", x=HD),
                    in1=dinv[:nb, :, None].to_broadcast([nb, H, HD]),
                )

                nc.tensor.transpose(
                    out=ps_epi[:, TR0 : TR0 + blk], in_=agg_s[:nb],
                    identity=ident[:nb, :nb],
                )
                aggT = epi.tile([D, blk], F16, tag="aggT")
                nc.scalar.activation(
                    out=aggT[:], in_=ps_epi[:, TR0 : TR0 + blk], func=AF.Copy
                )

                nc.tensor.matmul(
                    out=ps_epi[:nb, 0 : D + 1], lhsT=aggT[:, :nb], rhs=rhs_o[:],
                    start=True, stop=True,
                )
                nc.tensor.matmul(
                    out=ps_epi[:nb, D + 1 : 2 * D + 2],
                    lhsT=featT[:, b * blk : b * blk + nb],
                    rhs=rhs_s[:],
                    start=True, stop=True,
                )
                sk_s = epi.tile([blk, D + 1], F32, tag="sk")
                nc.scalar.activation(
                    out=sk_s[:nb], in_=ps_epi[:nb, D + 1 : 2 * D + 2], func=AF.Copy
                )
                gp = epi.tile([blk, 1], F32, tag="gp")
                nc.vector.tensor_add(
                    out=gp[:nb], in0=ps_epi[:nb, D : D + 1], in1=sk_s[:nb, 0:1]
                )
                g_s = epi.tile([blk, 1], F32, tag="g")
                nc.scalar.activation(
                    out=g_s[:nb], in_=gp[:nb], func=AF.Exp, scale=-1.0
                )
                nc.vector.tensor_scalar_add(out=g_s[:nb], in0=g_s[:nb], scalar1=1.0)
                nc.vector.reciprocal(out=g_s[:nb], in_=g_s[:nb])
                diff = epi.tile([blk, D], F32, tag="diff")
                nc.vector.tensor_sub(
                    out=diff[:nb], in0=ps_epi[:nb, 0:D], in1=sk_s[:nb, 1 : D + 1]
                )
                mix = epi.tile([blk, D], F32, tag="mix")
                nc.vector.scalar_tensor_tensor(
                    out=mix[:nb], in0=diff[:nb], scalar=g_s[:nb, 0:1],
                    in1=sk_s[:nb, 1 : D + 1],
                    op0=OP.mult, op1=OP.add,
                )

                h_t = epi.tile([blk, D], F32, tag="h")
                layer_norm(mix, 0, 1, h_t, nb)
                l2 = epi.tile([blk, D], F32, tag="l2")
                layer_norm(h_t, 2, 3, l2, nb)

                nc.tensor.transpose(
                    out=ps_epi[:, TR0 : TR0 + blk], in_=l2[:nb],
                    identity=ident[:nb, :nb],
                )
                l2T = epi.tile([D, blk], F16, tag="l2T")
                nc.scalar.activation(
                    out=l2T[:], in_=ps_epi[:, TR0 : TR0 + blk], func=AF.Copy
                )
                nc.tensor.matmul(
                    out=ps_epi[:nb, D + 2 : 2 * D + 2], lhsT=l2T[:, :nb],
                    rhs=w1_16[:],
                    start=True, stop=True,
                )
                r_t = epi.tile([blk, D], F32, tag="r")
                nc.scalar.activation(
                    out=r_t[:nb], in_=ps_epi[:nb, D + 2 : 2 * D + 2], func=AF.Relu
                )
                nc.tensor.transpose(
                    out=ps_epi[:, TR0 : TR0 + blk], in_=r_t[:nb],
                    identity=ident[:nb, :nb],
                )
                rT = epi.tile([D, blk], F16, tag="rT")
                nc.scalar.activation(
                    out=rT[:], in_=ps_epi[:, TR0 : TR0 + blk], func=AF.Copy
                )
                nc.tensor.matmul(
                    out=ps_epi[:nb, 0:D], lhsT=rT[:, :nb], rhs=w2_16[:],
                    start=True, stop=True,
                )
                outb = epi.tile([blk, D], F32, tag="outb")
                nc.vector.tensor_add(
                    out=outb[:nb], in0=h_t[:nb], in1=ps_epi[:nb, 0:D]
                )
                nc.sync.dma_start(
                    out=out_d[b * blk : b * blk + nb, :], in_=outb[:nb]
                )

    nc.compile()
    return nc


def _balance_blocks(deg, nblk, blk):
    """LPT-pack nodes into nblk blocks of exactly blk nodes, equalizing the
    per-block edge load. Returns newid[orig_local] -> new local id."""
    import heapq

    npc = len(deg)
    order = np.argsort(-deg, kind="stable")
    cnt = np.zeros(nblk, dtype=np.int64)
    heap = [(0, b) for b in range(nblk)]
    heapq.heapify(heap)
    newid = np.empty(npc, dtype=np.int64)
    for n in order:
        while True:
            load, b = heapq.heappop(heap)
            if cnt[b] < blk:
                break
        newid[n] = b * blk + cnt[b]
        cnt[b] += 1
        if cnt[b] < blk:
            heapq.heappush(heap, (load + deg[n], b))
    return newid


def compute_layout(inputs, base):
    """Permute nodes within each core so per-block edge loads are balanced
    (lower static block capacity M), then lay edges out by dst block."""
    cores, npc, nblk, blk = base["cores"], base["npc"], base["nblk"], base["blk"]
    nblk_g = cores * nblk

    src = np.asarray(inputs["src"]).astype(np.int64)
    dst = np.asarray(inputs["dst"]).astype(np.int64)

    # per-core node permutation (new local id = block*blk + slot)
    newid = np.empty(cores * npc, dtype=np.int64)
    for c in range(cores):
        deg = np.bincount(dst[(dst >= c * npc) & (dst < (c + 1) * npc)] - c * npc,
                          minlength=npc)
        newid[c * npc : (c + 1) * npc] = c * npc + _balance_blocks(deg, nblk, blk)

    dstp = newid[dst]
    gb_all = dstp // blk
    order = np.lexsort((src, gb_all))
    ds = dstp[order]
    ss = src[order]
    gb = gb_all[order]

    counts = np.bincount(gb, minlength=nblk_g)
    M = max(int(os.environ.get("KERNEL_MIN_M", "2")),
            int(np.ceil(counts.max() / 128)))

    starts = np.zeros(nblk_g + 1, dtype=np.int64)
    np.cumsum(counts, out=starts[1:])
    pos = np.arange(len(ds)) - starts[gb]
    slot = gb * (M * 128) + pos

    layout = dict(order=order, ds=ds, ss=ss, gb=gb, slot=slot, newid=newid)
    cfg = dict(base, M=M)
    return cfg, layout


def shard_inputs(inputs, cfg, layout):
    """Host-side layout only (sort/pad/transpose/index/cast, no arithmetic)."""
    cores = cfg["cores"]
    npc = cfg["npc"]
    nblk = cfg["nblk"]
    blk = cfg["blk"]
    M = cfg["M"]
    CAP = M * 128
    nblk_g = cores * nblk

    ds, ss, slot = layout["ds"], layout["ss"], layout["slot"]
    gb = layout["gb"]
    edge_feat = np.asarray(inputs["edge_feat"])
    feat = np.asarray(inputs["feat"])
    featp = np.empty_like(feat)
    featp[layout["newid"]] = feat

    total = nblk_g * CAP
    dstloc = np.full(total, blk, dtype=np.int64)
    dstloc[slot] = ds - gb * blk

    ef_pad = np.zeros((total, D), dtype=np.float16)
    ef_pad[slot] = edge_feat[layout["order"]]
    fs_pad = np.zeros((total, D), dtype=np.float16)
    fs_pad[slot] = feat[ss]

    f8 = mybir.dt.np(F8)
    sb_ = np.arange(total) % CAP
    gb_s = np.arange(total) // CAP
    # ohT[b, n, j, e] = 1 iff dst_local(slot j*128+e of block b) == n
    ohT = np.zeros(nblk_g * 128 * CAP, dtype=f8)
    ohT_idx = ((gb_s * 128 + dstloc) * (CAP // 128) + sb_ // 128) * 128 + sb_ % 128
    ohT[ohT_idx] = 1.0
    ohT = ohT.reshape(nblk_g, 128, CAP // 128, 128)
    # oh[b, e, j, n] = 1 iff dst_local(slot j*128+e of block b) == n
    # pads (dstloc==blk==125) land in discarded output rows 125..127
    oh = np.zeros(nblk_g * 128 * CAP, dtype=f8)
    oh_idx = ((gb_s * 128 + sb_ % 128) * (CAP // 128) + sb_ // 128) * 128 + dstloc
    oh[oh_idx] = 1.0
    oh = oh.reshape(nblk_g, 128, CAP // 128, 128)

    per_core = nblk * CAP
    in_maps = []
    for c_i in range(cores):
        bsl = slice(c_i * nblk, (c_i + 1) * nblk)
        sl = slice(c_i * per_core, (c_i + 1) * per_core)
        m = {
            "fsT": np.ascontiguousarray(fs_pad[sl].T),
            "efT": np.ascontiguousarray(ef_pad[sl].T),
            "ohT": np.ascontiguousarray(ohT[bsl]),
            "oh": np.ascontiguousarray(oh[bsl]),
            "featT": np.ascontiguousarray(
                featp[c_i * npc : (c_i + 1) * npc].T.astype(np.float16)
            ),
            "WoT": np.ascontiguousarray(np.asarray(inputs["Wo"]).T),
            "WskipT": np.ascontiguousarray(np.asarray(inputs["Wskip"]).T),
            "Weg16": np.ascontiguousarray(
                np.concatenate(
                    [np.asarray(inputs["We"]), np.asarray(inputs["Wg"])], axis=1
                ).astype(np.float16)
            ),
            "Wgate": np.ascontiguousarray(np.asarray(inputs["Wgate"])),
        }
        for name in ("Wq", "Wk", "Wv", "Wo", "Wskip", "W1", "W2"):
            m[name + "16"] = np.ascontiguousarray(
                np.asarray(inputs[name]).astype(np.float16)
            )
        for name in ("ln1_g", "ln1_b", "ln2_g", "ln2_b"):
            m[name] = np.ascontiguousarray(np.asarray(inputs[name]))
        in_maps.append(m)
    return in_maps


_cache = {}


def _get_program(cfg):
    key = (cfg["cores"], cfg["n_nodes"], cfg["M"], TREE_GPS, GPS_PW, os.environ.get("KERNEL_DBG","0"))
    if key not in _cache:
        _cache[key] = build_program(cfg)
    return _cache[key]


def full_base():
    return dict(cores=CORES, n_nodes=N_NODES, npc=NPC, nblk=NBLK, blk=BLK)


def _ensure_ntff_hook():
    import types

    if "antenv.axon_hooks" in sys.modules:
        return
    try:
        sys.path.insert(0, "/root/.axon_site")
        from trn_agent_boot.trn_boot import _ntff_profile_via_ctypes

        hook = _ntff_profile_via_ctypes("/opt/axon/libaxon_pjrt.so")
        mod = types.ModuleType("antenv.axon_hooks")
        mod.get_axon_ntff_profile_hook = lambda: hook
        mod.set_axon_ntff_profile_hook = lambda h: None
        sys.modules["antenv.axon_hooks"] = mod
    except Exception as e:
        print(f"ntff hook setup failed: {e}")


def run(inputs, trace=False, tmpdir=None, trace_cores=None):
    if trace:
        _ensure_ntff_hook()
    cfg, layout = compute_layout(inputs, full_base())
    nc = _get_program(cfg)
    in_maps = shard_inputs(inputs, cfg, layout)
    res = bass_utils.run_bass_kernel_spmd(
        nc,
        in_maps,
        core_ids=list(range(cfg["cores"])),
        trace=trace,
        tmpdir=tmpdir,
        trace_cores=trace_cores,
    )
    out = np.concatenate([res.results[c]["out"] for c in range(cfg["cores"])], axis=0)
    out = out[layout["newid"]]
    return out, res


def kernel(**inputs):
    out, _ = run(inputs)
    return out
